# revision 11
# baseline (speedup 1.0000x reference)
"""Trainium2 Bass kernel for the arc-projection problem.

Full-input contract: kernel(**inputs) takes the unsharded numpy inputs and
returns the full output. Internally shards the batch N=64 across 8 cores
(pure data parallel), runs one SPMD Bass kernel, and gathers.

Algorithm (matches reference._arc_projection, reformulated gather-free):
  For each (sample, branch, direction) row:
    - segment vectors sv, masked lengths sl, cumsum cum, masked unit
      tangents wm = sv/sl*mask, and their first differences dw.
    - project trajectory point 0 on all segments -> entry_s (one-hot argmin)
    - target_s[t] = clip(entry_s + traj_cum[t], 0, total)
    - proj_c(s) = base_c + sum_j dw_c[j] * relu(s - cum_j)   (telescoped
      hinge; equals the reference's searchsorted+lerp for prefix/suffix
      masks). Computed by a custom fused DVE op: one instruction per t
      producing running sums whose page-ends give all 3 channels.
    - cost = sum_t |pos_t - proj_t|; per-sample argmin over 32 rows via
      one-hot; best-branch gather via a PE matmul with a block-diagonal
      one-hot stationary (no DRAM round-trip).
"""

import sys

import numpy as np

try:
    import concourse.bass as bass
except ImportError:  # pragma: no cover - container without PYTHONPATH set
    sys.path.insert(0, "/opt/trn_rl_repo")
    import concourse.bass as bass

import concourse.tile as tile
from concourse import bacc, mybir
from concourse.bass_utils import run_bass_kernel_spmd

import concourse.dve_ops as _dmod
from concourse.dve_spec import (
    Spec, Src0, Src1, C0, relu, scan,
    AluOp as _AluOp, lower as _dve_lower, _has_src1,
)
from concourse.dve_uop import DveOpSpec as _DveOpSpec

f32 = mybir.dt.float32
AT = mybir.AluOpType
AX = mybir.AxisListType

N, T, NB, NP = 64, 128, 16, 256
NCORES = 8
NS = N // NCORES          # samples per core
NB2 = 2 * NB              # fwd + bwd branches
NSEG = NP - 1
NS2 = NP                  # padded segment count (256) for the hinge stream
BIG = 1.0e30
RT = 128                  # rows per partition-tile
GP_T = 0                  # Pool offload disabled: SBUF-port contention with DVE


def _register_hinge_op():
    """out[p,k] = cumsum_k(relu(in0[p,k] + s0[p]) * in1[p,k]) as one DVE op."""
    name = "HINGE_SCAN_ANT"
    for o in _dmod.OPS:
        if o.name == name:
            return o

    def _ref(in0, in1, s0, s1, imm2):
        x = (np.maximum(in0.astype(np.float32) + s0, 0.0) * in1).astype(np.float32)
        xf = x.reshape(x.shape[0], -1)
        return np.cumsum(xf, axis=-1).astype(np.float32).reshape(x.shape)

    spec = Spec(body=scan(_AluOp.ADD, relu(Src0 + C0) * Src1), reference=_ref)
    op = _dmod.DveOp(name, spec, subdim=False, uops_sha={})
    _dmod.OPS.append(op)
    _dmod._SUB_OPCODE_FOR_NAME[name] = _dmod._CUSTOM_DVE_ROW_BASE + len(_dmod.OPS) - 1
    for ver in ("v3", "v4"):
        _dmod._COMPILE_CACHE[(name, ver)] = _DveOpSpec(
            name=name,
            opcode=_dmod._SUB_OPCODE_FOR_NAME[name],
            uops=_dve_lower(spec, ver=ver),
            rd1_en=_has_src1(spec),
        )
    return op


HINGE = _register_hinge_op()


def _register_diffsq_op():
    """out[p,k] = (in0[p,k] - in1[p,k])**2 as one DVE op."""
    name = "DIFFSQ_ANT"
    for o in _dmod.OPS:
        if o.name == name:
            return o
    from concourse.dve_spec import sq as _sq
    spec = Spec(
        body=_sq(Src0 - Src1),
        reference=lambda in0, in1, s0, s1, imm2:
            ((in0.astype(np.float32) - in1) ** 2).astype(np.float32),
    )
    op = _dmod.DveOp(name, spec, subdim=False, uops_sha={})
    _dmod.OPS.append(op)
    _dmod._SUB_OPCODE_FOR_NAME[name] = _dmod._CUSTOM_DVE_ROW_BASE + len(_dmod.OPS) - 1
    for ver in ("v3", "v4"):
        _dmod._COMPILE_CACHE[(name, ver)] = _DveOpSpec(
            name=name,
            opcode=_dmod._SUB_OPCODE_FOR_NAME[name],
            uops=_dve_lower(spec, ver=ver),
            rd1_en=_has_src1(spec),
        )
    return op


DIFFSQ = _register_diffsq_op()


def _view(t, ap_dims, extra_off=0):
    """Strided view of a tile/AP: ap_dims are [step, count] free dims after
    the partition dim (kept from t)."""
    return bass.AP(tensor=t.tensor, offset=t.offset + extra_off,
                   ap=[t.ap[0]] + ap_dims)


def _dview(t, ap_dims, extra_off=0):
    """Raw view of a DRAM tile: ap_dims replace all dims."""
    return bass.AP(tensor=t.tensor, offset=t.offset + extra_off, ap=ap_dims)


def build_nc(ns=NS, enable_asserts=False):
    rows = ns * NB2
    ntiles = (rows + RT - 1) // RT
    spt = RT // NB2  # samples per tile

    nc = bacc.Bacc("TRN2", target_bir_lowering=False, debug=False,
                   enable_asserts=enable_asserts, num_devices=NCORES)

    rp_d = nc.dram_tensor("rp", [rows, 3, NP], f32, kind="ExternalInput")
    mk_d = nc.dram_tensor("mk", [rows, NP], f32, kind="ExternalInput")
    tj_d = nc.dram_tensor("tj", [ns, 3, T], f32, kind="ExternalInput")
    m4_d = nc.dram_tensor("m4", [RT, spt], f32, kind="ExternalInput")
    out_d = nc.dram_tensor("out", [ns, T, 3], f32, kind="ExternalOutput")

    with tile.TileContext(nc) as tc:
        with (
            tc.tile_pool(name="work", bufs=2) as wp,
            tc.tile_pool(name="fin", bufs=1) as fp,
            tc.tile_pool(name="ps", bufs=1, space="PSUM") as pp,
            tc.tile_pool(name="dram", bufs=1, space="DRAM") as dp,
        ):
            cost_s = dp.tile([rows], f32)
            oh_s = dp.tile([rows], f32)
            mask4 = fp.tile([RT, spt], f32)
            nc.sync.dma_start(out=mask4, in_=m4_d.ap())

            projs = []
            for k in range(ntiles):
                p = min(RT, rows - k * RT)
                r0 = k * RT

                rpt = wp.tile([p, 3, NP], f32, tag="rpt")
                nc.sync.dma_start(out=rpt, in_=rp_d.ap()[r0:r0 + p])
                mt = wp.tile([p, NP], f32, tag="mt")
                nc.sync.dma_start(out=mt, in_=mk_d.ap()[r0:r0 + p])
                # trajectory of each row's sample, broadcast to its 32 rows
                tpb = wp.tile([p, 3, T], f32, tag="tpb")
                nc.sync.dma_start(out=tpb, in_=bass.AP(
                    tensor=tj_d.ap().tensor, offset=k * spt * 3 * T,
                    ap=[[3 * T, spt], [0, NB2], [1, 3 * T]]))

                # --- segment data ---
                sv = wp.tile([p, 3, NSEG], f32, tag="sv")
                nc.vector.tensor_sub(out=sv, in0=rpt[:, :, 1:NP],
                                     in1=rpt[:, :, 0:NSEG])
                sm = wp.tile([p, NSEG], f32, tag="sm")
                nc.vector.tensor_mul(out=sm, in0=mt[:, 1:NP], in1=mt[:, 0:NSEG])
                sq3 = wp.tile([p, 3, NSEG], f32, tag="sq3")
                nc.vector.tensor_mul(out=sq3, in0=sv, in1=sv)
                sl2 = wp.tile([p, NSEG], f32, tag="sl2")
                nc.vector.tensor_reduce(out=sl2,
                                        in_=_view(sq3, [[1, NSEG], [NSEG, 3]]),
                                        axis=AX.X, op=AT.add)
                sl2m = wp.tile([p, NSEG], f32, tag="sl2m")
                nc.vector.tensor_mul(out=sl2m, in0=sl2, in1=sm)
                sl = wp.tile([p, NSEG], f32, tag="sl")
                eps2 = wp.tile([p, 1], f32, tag="eps2")
                nc.vector.memset(eps2, 1e-18)
                nc.scalar.activation(out=sl, in_=sl2m,
                                     func=mybir.ActivationFunctionType.Sqrt,
                                     bias=eps2)

                cum = wp.tile([p, NP], f32, tag="cum")
                zc = wp.tile([p, 1], f32, tag="zc")
                nc.vector.memset(zc, 0.0)
                nc.vector.memset(cum[:, 0:1], 0.0)
                nc.vector.tensor_tensor_scan(
                    out=cum[:, 1:NP], data0=sl, data1=_view(zc, [[0, NSEG]]),
                    initial=0.0, op0=AT.add, op1=AT.add)
                total = cum[:, NP - 1:NP]
                # negated cumsum for the hinge stream (on ACT, frees DVE)
                cumneg = wp.tile([p, NSEG], f32, tag="cumneg")
                nc.scalar.activation(out=cumneg, in_=cum[:, 0:NSEG],
                                     func=mybir.ActivationFunctionType.Copy,
                                     scale=-1.0)
                rsl = wp.tile([p, NSEG], f32, tag="rsl")
                rscr = wp.tile([p, NSEG], f32, tag="rscr")
                nc.vector.reciprocal_approx_accurate(out=rsl, in_=sl,
                                                     scratch=rscr)
                rslm = wp.tile([p, NSEG], f32, tag="rslm")
                nc.vector.tensor_mul(out=rslm, in0=rsl, in1=sm)
                wm = wp.tile([p, 3, NSEG], f32, tag="wm")
                nc.vector.tensor_mul(out=wm, in0=sv,
                                     in1=_view(rslm, [[0, 3], [1, NSEG]]))
                # dw[c,0] = wm[c,0]; dw[c,j] = wm[c,j]-wm[c,j-1]
                dw = wp.tile([p, 3, NSEG], f32, tag="dw")
                nc.vector.tensor_copy(out=_view(dw, [[NSEG, 3]]),
                                      in_=_view(wm, [[NSEG, 3]]))
                nc.vector.tensor_sub(
                    out=_view(dw, [[NSEG, 3], [1, NSEG - 1]], extra_off=1),
                    in0=_view(wm, [[NSEG, 3], [1, NSEG - 1]], extra_off=1),
                    in1=_view(wm, [[NSEG, 3], [1, NSEG - 1]]))

                # --- project p0 on all segments; entry_s via one-hot argmin ---
                tmp3 = wp.tile([p, 3, NSEG], f32, tag="tmp3")
                for c in range(3):
                    # (a_c - p0_c) * sv_c
                    nc.vector.scalar_tensor_tensor(
                        out=tmp3[:, c, :], in0=rpt[:, c, 0:NSEG],
                        scalar=tpb[:, c, 0:1], in1=sv[:, c, :],
                        op0=AT.subtract, op1=AT.mult)
                dotn = wp.tile([p, NSEG], f32, tag="dotn")
                nc.vector.tensor_reduce(out=dotn,
                                        in_=_view(tmp3, [[1, NSEG], [NSEG, 3]]),
                                        axis=AX.X, op=AT.add)
                svd = wp.tile([p, NSEG], f32, tag="svd")
                nc.vector.tensor_scalar(out=svd, in0=sl2, scalar1=1e-12,
                                        scalar2=None, op0=AT.max)
                rsvd = wp.tile([p, NSEG], f32, tag="rsvd")
                nc.vector.reciprocal_approx_accurate(out=rsvd, in_=svd,
                                                     scratch=rscr)
                t0 = wp.tile([p, NSEG], f32, tag="t0")
                nc.vector.tensor_mul(out=t0, in0=dotn, in1=rsvd)
                # t0 = min(max(-t0, 0), 1)
                nc.vector.tensor_scalar(out=t0, in0=t0, scalar1=-1.0,
                                        scalar2=0.0, op0=AT.mult, op1=AT.max)
                nc.vector.tensor_scalar(out=t0, in0=t0, scalar1=1.0,
                                        scalar2=None, op0=AT.min)
                s3 = wp.tile([p, 3, NSEG], f32, tag="s3")
                nc.vector.tensor_mul(out=s3, in0=sv,
                                     in1=_view(t0, [[0, 3], [1, NSEG]]))
                e3 = wp.tile([p, 3, NSEG], f32, tag="e3")
                for c in range(3):
                    # (a_c - p0_c) + t0*sv_c  (= q0_c - p0_c)
                    nc.vector.scalar_tensor_tensor(
                        out=e3[:, c, :], in0=rpt[:, c, 0:NSEG],
                        scalar=tpb[:, c, 0:1], in1=s3[:, c, :],
                        op0=AT.subtract, op1=AT.add)
                e3sq = wp.tile([p, 3, NSEG], f32, tag="e3sq")
                nc.vector.tensor_mul(out=e3sq, in0=e3, in1=e3)
                d2 = wp.tile([p, NSEG], f32, tag="d2")
                nc.vector.tensor_reduce(out=d2,
                                        in_=_view(e3sq, [[1, NSEG], [NSEG, 3]]),
                                        axis=AX.X, op=AT.add)
                d2m = wp.tile([p, NSEG], f32, tag="d2m")
                # d2m = d2 + (1-sm)*BIG  (sm is exactly 0/1)
                nc.vector.tensor_scalar(out=d2m, in0=sm, scalar1=1.0,
                                        scalar2=-BIG, op0=AT.subtract,
                                        op1=AT.mult)
                nc.vector.tensor_add(out=d2m, in0=d2m, in1=d2)
                dmin = wp.tile([p, 1], f32, tag="dmin")
                nc.vector.tensor_reduce(out=dmin, in_=d2m, axis=AX.X, op=AT.min)
                ohseg = wp.tile([p, NSEG], f32, tag="ohseg")
                nc.vector.tensor_scalar(out=ohseg, in0=d2m, scalar1=dmin,
                                        scalar2=None, op0=AT.is_equal)
                # keep only the FIRST hot via prefix-max diff (jnp.argmin ties)
                pmax = wp.tile([p, NSEG], f32, tag="pmax")
                nc.vector.tensor_tensor_scan(
                    out=pmax, data0=ohseg, data1=_view(zc, [[0, NSEG]]),
                    initial=0.0, op0=AT.max, op1=AT.add)
                nc.vector.tensor_copy(out=ohseg[:, 0:1], in_=pmax[:, 0:1])
                nc.vector.tensor_sub(out=ohseg[:, 1:NSEG], in0=pmax[:, 1:NSEG],
                                     in1=pmax[:, 0:NSEG - 1])
                es = wp.tile([p, NSEG], f32, tag="es")
                nc.vector.tensor_mul(out=es, in0=t0, in1=sl)
                nc.vector.tensor_add(out=es, in0=es, in1=cum[:, 0:NSEG])
                entry = wp.tile([p, 1], f32, tag="entry")
                junk0 = wp.tile([p, NSEG], f32, tag="junk0")
                nc.vector.scalar_tensor_tensor(
                    out=junk0, in0=ohseg, scalar=1.0, in1=es,
                    op0=AT.mult, op1=AT.mult, accum_out=entry)

                # --- base point rp[first valid segment] ---
                ohf = wp.tile([p, NSEG], f32, tag="ohf")
                nc.vector.tensor_copy(out=ohf[:, 0:1], in_=sm[:, 0:1])
                nc.vector.tensor_sub(out=ohf[:, 1:NSEG], in0=sm[:, 1:NSEG],
                                     in1=sm[:, 0:NSEG - 1])
                nc.vector.tensor_scalar(out=ohf, in0=ohf, scalar1=0.0,
                                        scalar2=None, op0=AT.max)
                base3 = wp.tile([p, 3], f32, tag="base3")
                for c in range(3):
                    nc.vector.scalar_tensor_tensor(
                        out=junk0, in0=ohf, scalar=1.0, in1=rpt[:, c, 0:NSEG],
                        op0=AT.mult, op1=AT.mult,
                        accum_out=base3[:, c:c + 1])

                # --- trajectory cumulative arc length + target_s ---
                td = wp.tile([p, 3, T - 1], f32, tag="td")
                nc.vector.tensor_sub(out=td, in0=tpb[:, :, 1:T],
                                     in1=tpb[:, :, 0:T - 1])
                td2 = wp.tile([p, 3, T - 1], f32, tag="td2")
                nc.vector.tensor_mul(out=td2, in0=td, in1=td)
                tl2 = wp.tile([p, T - 1], f32, tag="tl2")
                nc.vector.tensor_reduce(out=tl2,
                                        in_=_view(td2, [[1, T - 1], [T - 1, 3]]),
                                        axis=AX.X, op=AT.add)
                tl = wp.tile([p, T - 1], f32, tag="tl")
                nc.scalar.sqrt(out=tl, in_=tl2)
                tcum = wp.tile([p, T], f32, tag="tcum")
                nc.vector.memset(tcum[:, 0:1], 0.0)
                nc.vector.tensor_tensor_scan(
                    out=tcum[:, 1:T], data0=tl, data1=_view(zc, [[0, T - 1]]),
                    initial=0.0, op0=AT.add, op1=AT.add)
                target = wp.tile([p, T], f32, tag="target")
                nc.vector.scalar_tensor_tensor(
                    out=target, in0=tcum, scalar=entry,
                    in1=_view(total, [[0, T]]), op0=AT.add, op1=AT.min)
                nc.vector.tensor_scalar(out=target, in0=target, scalar1=0.0,
                                        scalar2=None, op0=AT.max)

                # --- main pass: fused hinge-scan, one DVE instr per t;
                # the last GP_T timesteps run on Pool (gpsimd) + ACT accum ---
                TD = T - GP_T
                E = wp.tile([p, 3, T], f32, tag="E")
                scrs = [wp.tile([p, 3 * NSEG], f32, name=f"scr{k}_{i}")
                        for i in range(3)]
                cn_b = _view(cumneg, [[0, 3], [1, NSEG]])
                if GP_T > 0:
                    Eg = wp.tile([p, 3, GP_T], f32, tag="Eg")
                    vts = [wp.tile([p, NSEG], f32, name=f"vt{k}_{i}")
                           for i in range(2)]
                    mcs = [wp.tile([p, NSEG], f32, name=f"mc{k}_{i}")
                           for i in range(6)]
                for t in range(T):
                    if t < TD:
                        scr = scrs[t % 3]
                        nc.vector._custom_dve(
                            HINGE, out=scr, in0=cn_b, in1=dw,
                            s0=target[:, t:t + 1], s1=0.0)
                        nc.scalar.copy(
                            out=_view(E, [[T, 3]], extra_off=t),
                            in_=_view(scr, [[NSEG, 3]], extra_off=NSEG - 1))
                    else:
                        i = t - TD
                        vt = vts[i % 2]
                        nc.gpsimd.tensor_scalar(
                            out=vt, in0=cumneg, scalar1=target[:, t:t + 1],
                            scalar2=0.0, op0=AT.add, op1=AT.max)
                        for c in range(3):
                            mc = mcs[(3 * i + c) % 6]
                            nc.gpsimd.tensor_mul(out=mc, in0=vt,
                                                 in1=dw[:, c, :])
                            nc.scalar.activation(
                                out=mc, in_=mc,
                                func=mybir.ActivationFunctionType.Copy,
                                accum_out=Eg[:, c, i:i + 1])

                # page-end diffs + base -> proj
                proj = wp.tile([p, 3, T], f32, name=f"proj{k}", bufs=1)
                nc.vector.tensor_scalar(out=proj[:, 0, 0:TD], in0=E[:, 0, 0:TD],
                                        scalar1=base3[:, 0:1], scalar2=None,
                                        op0=AT.add)
                for c in (1, 2):
                    nc.vector.scalar_tensor_tensor(
                        out=proj[:, c, 0:TD], in0=E[:, c, 0:TD],
                        scalar=base3[:, c:c + 1], in1=E[:, c - 1, 0:TD],
                        op0=AT.add, op1=AT.subtract)
                if GP_T > 0:
                    for c in range(3):
                        nc.vector.tensor_scalar(
                            out=proj[:, c, TD:T], in0=Eg[:, c, :],
                            scalar1=base3[:, c:c + 1], scalar2=None,
                            op0=AT.add)
                projs.append(proj)

                # --- cost ---
                df2 = wp.tile([p, 3, T], f32, tag="df2")
                nc.vector._custom_dve(DIFFSQ, out=df2, in0=proj, in1=tpb,
                                      s0=0.0, s1=0.0)
                dd = wp.tile([p, T], f32, tag="dd")
                nc.vector.tensor_reduce(out=dd,
                                        in_=_view(df2, [[1, T], [T, 3]]),
                                        axis=AX.X, op=AT.add)
                dist = wp.tile([p, T], f32, tag="dist")
                cost = wp.tile([p, 1], f32, tag="cost")
                nc.scalar.activation(out=dist, in_=dd,
                                     func=mybir.ActivationFunctionType.Sqrt,
                                     accum_out=cost)
                nc.sync.dma_start(
                    out=_dview(cost_s, [[1, p]], extra_off=r0), in_=cost)

                # --- per-sample argmin over this tile's branches ---
                costT = fp.tile([spt, NB2], f32, name=f"costT{k}")
                nc.sync.dma_start(
                    out=costT,
                    in_=_dview(cost_s, [[NB2, spt], [1, NB2]], extra_off=r0))
                cmin = fp.tile([spt, 1], f32, name=f"cmin{k}")
                nc.vector.tensor_reduce(out=cmin, in_=costT, axis=AX.X,
                                        op=AT.min)
                oh8 = fp.tile([spt, NB2], f32, name=f"oh8{k}")
                nc.vector.tensor_scalar(out=oh8, in0=costT, scalar1=cmin,
                                        scalar2=None, op0=AT.is_equal)
                zc8 = fp.tile([spt, 1], f32, name=f"zc8{k}")
                nc.vector.memset(zc8, 0.0)
                pm8 = fp.tile([spt, NB2], f32, name=f"pm8{k}")
                nc.vector.tensor_tensor_scan(
                    out=pm8, data0=oh8, data1=_view(zc8, [[0, NB2]]),
                    initial=0.0, op0=AT.max, op1=AT.add)
                nc.vector.tensor_copy(out=oh8[:, 0:1], in_=pm8[:, 0:1])
                nc.vector.tensor_sub(out=oh8[:, 1:NB2], in0=pm8[:, 1:NB2],
                                     in1=pm8[:, 0:NB2 - 1])
                nc.sync.dma_start(
                    out=_dview(oh_s, [[NB2, spt], [1, NB2]], extra_off=r0),
                    in_=oh8)

                # --- best-branch gather via PE: block-diag one-hot matmul ---
                ohcol = fp.tile([p, 1], f32, name=f"ohcol{k}")
                nc.sync.dma_start(out=ohcol,
                                  in_=_dview(oh_s, [[1, p]], extra_off=r0))
                ohbd = fp.tile([p, spt], f32, name=f"ohbd{k}")
                nc.vector.tensor_scalar(out=ohbd, in0=mask4, scalar1=ohcol,
                                        scalar2=None, op0=AT.mult)
                pj = pp.tile([spt, 3 * T], f32, name=f"pj{k}")
                nc.tensor.matmul(pj, ohbd, projs[k], start=True, stop=True)
                outt = fp.tile([spt, T, 3], f32, name=f"outt{k}")
                nc.scalar.copy(out=_view(outt, [[1, 3], [3, T]]),
                               in_=_view(pj, [[T, 3], [1, T]]))
                nc.sync.dma_start(out=out_d.ap()[k * spt:(k + 1) * spt],
                                  in_=outt)

    nc.compile()
    return nc


def marshal_inputs(selected_traj, road_points, road_mask):
    """Host-side layout marshaling (permutations/casts only): per-core input
    dicts with fwd+bwd branch rows and planar (xyz-major) layouts."""
    st = np.ascontiguousarray(selected_traj, dtype=np.float32)
    rp = np.ascontiguousarray(road_points, dtype=np.float32)
    rm = np.asarray(road_mask)

    rp_ext = np.concatenate([rp, rp[:, :, ::-1, :]], axis=1)        # [N,NB2,NP,3]
    rp_ext = np.ascontiguousarray(rp_ext.transpose(0, 1, 3, 2))     # [N,NB2,3,NP]
    mk_ext = np.concatenate([rm, rm[:, :, ::-1]], axis=1).astype(np.float32)
    tj = np.ascontiguousarray(st.transpose(0, 2, 1))                # [N,3,T]

    spt = RT // NB2
    m4 = np.zeros((RT, spt), dtype=np.float32)
    for s in range(spt):
        m4[s * NB2:(s + 1) * NB2, s] = 1.0

    in_maps = []
    for c in range(NCORES):
        s = slice(c * NS, (c + 1) * NS)
        in_maps.append({
            "rp": np.ascontiguousarray(rp_ext[s]).reshape(NS * NB2, 3, NP),
            "mk": np.ascontiguousarray(mk_ext[s]).reshape(NS * NB2, NP),
            "tj": np.ascontiguousarray(tj[s]),
            "m4": m4,
        })
    return in_maps


_NC = None


def kernel(selected_traj, road_points, road_mask):
    global _NC
    if _NC is None:
        _NC = build_nc()
    in_maps = marshal_inputs(selected_traj, road_points, road_mask)
    res = run_bass_kernel_spmd(_NC, in_maps, core_ids=list(range(NCORES)))
    out = np.concatenate([r["out"] for r in res.results], axis=0)
    return out.astype(np.float32)


# revision 12
# speedup vs baseline: 1.0149x; 1.0149x over previous
"""Trainium2 Bass kernel for the arc-projection problem.

Full-input contract: kernel(**inputs) takes the unsharded numpy inputs and
returns the full output. Internally shards the batch N=64 across 8 cores
(pure data parallel), runs one SPMD Bass kernel, and gathers.

Algorithm (matches reference._arc_projection, reformulated gather-free):
  For each (sample, branch, direction) row:
    - segment vectors sv, masked lengths sl, cumsum cum, masked unit
      tangents wm = sv/sl*mask, and their first differences dw.
    - project trajectory point 0 on all segments -> entry_s (one-hot argmin)
    - target_s[t] = clip(entry_s + traj_cum[t], 0, total)
    - proj_c(s) = base_c + sum_j dw_c[j] * relu(s - cum_j)   (telescoped
      hinge; equals the reference's searchsorted+lerp for prefix/suffix
      masks). Computed by a custom fused DVE op: one instruction per t
      producing running sums whose page-ends give all 3 channels.
    - cost = sum_t |pos_t - proj_t|; per-sample argmin over 32 rows via
      one-hot; best-branch gather via a PE matmul with a block-diagonal
      one-hot stationary (no DRAM round-trip).
"""

import sys

import numpy as np

try:
    import concourse.bass as bass
except ImportError:  # pragma: no cover - container without PYTHONPATH set
    sys.path.insert(0, "/opt/trn_rl_repo")
    import concourse.bass as bass

import concourse.tile as tile
from concourse import bacc, mybir
from concourse.bass_utils import run_bass_kernel_spmd

import concourse.dve_ops as _dmod
from concourse.dve_spec import (
    Spec, Src0, Src1, C0, relu, scan,
    AluOp as _AluOp, lower as _dve_lower, _has_src1,
)
from concourse.dve_uop import DveOpSpec as _DveOpSpec

f32 = mybir.dt.float32
AT = mybir.AluOpType
AX = mybir.AxisListType

N, T, NB, NP = 64, 128, 16, 256
NCORES = 8
NS = N // NCORES          # samples per core
NB2 = 2 * NB              # fwd + bwd branches
NSEG = NP - 1
NS2 = NP                  # padded segment count (256) for the hinge stream
BIG = 1.0e30
RT = 128                  # rows per partition-tile
GP_T = 0                  # Pool offload disabled: SBUF-port contention with DVE


def _register_hinge_op():
    """out[p,k] = cumsum_k(relu(in0[p,k] + s0[p]) * in1[p,k]) as one DVE op."""
    name = "HINGE_SCAN_ANT"
    for o in _dmod.OPS:
        if o.name == name:
            return o

    def _ref(in0, in1, s0, s1, imm2):
        x = (np.maximum(in0.astype(np.float32) + s0, 0.0) * in1).astype(np.float32)
        xf = x.reshape(x.shape[0], -1)
        return np.cumsum(xf, axis=-1).astype(np.float32).reshape(x.shape)

    spec = Spec(body=scan(_AluOp.ADD, relu(Src0 + C0) * Src1), reference=_ref)
    op = _dmod.DveOp(name, spec, subdim=False, uops_sha={})
    _dmod.OPS.append(op)
    _dmod._SUB_OPCODE_FOR_NAME[name] = _dmod._CUSTOM_DVE_ROW_BASE + len(_dmod.OPS) - 1
    for ver in ("v3", "v4"):
        _dmod._COMPILE_CACHE[(name, ver)] = _DveOpSpec(
            name=name,
            opcode=_dmod._SUB_OPCODE_FOR_NAME[name],
            uops=_dve_lower(spec, ver=ver),
            rd1_en=_has_src1(spec),
        )
    return op


HINGE = _register_hinge_op()


def _register_diffsq_op():
    """out[p,k] = (in0[p,k] - in1[p,k])**2 as one DVE op."""
    name = "DIFFSQ_ANT"
    for o in _dmod.OPS:
        if o.name == name:
            return o
    from concourse.dve_spec import sq as _sq
    spec = Spec(
        body=_sq(Src0 - Src1),
        reference=lambda in0, in1, s0, s1, imm2:
            ((in0.astype(np.float32) - in1) ** 2).astype(np.float32),
    )
    op = _dmod.DveOp(name, spec, subdim=False, uops_sha={})
    _dmod.OPS.append(op)
    _dmod._SUB_OPCODE_FOR_NAME[name] = _dmod._CUSTOM_DVE_ROW_BASE + len(_dmod.OPS) - 1
    for ver in ("v3", "v4"):
        _dmod._COMPILE_CACHE[(name, ver)] = _DveOpSpec(
            name=name,
            opcode=_dmod._SUB_OPCODE_FOR_NAME[name],
            uops=_dve_lower(spec, ver=ver),
            rd1_en=_has_src1(spec),
        )
    return op


DIFFSQ = _register_diffsq_op()


def _view(t, ap_dims, extra_off=0):
    """Strided view of a tile/AP: ap_dims are [step, count] free dims after
    the partition dim (kept from t)."""
    return bass.AP(tensor=t.tensor, offset=t.offset + extra_off,
                   ap=[t.ap[0]] + ap_dims)


def _dview(t, ap_dims, extra_off=0):
    """Raw view of a DRAM tile: ap_dims replace all dims."""
    return bass.AP(tensor=t.tensor, offset=t.offset + extra_off, ap=ap_dims)


def build_nc(ns=NS, enable_asserts=False):
    rows = ns * NB2
    ntiles = (rows + RT - 1) // RT
    spt = RT // NB2  # samples per tile

    nc = bacc.Bacc("TRN2", target_bir_lowering=False, debug=False,
                   enable_asserts=enable_asserts, num_devices=NCORES)

    rp_d = nc.dram_tensor("rp", [rows, 3, NP], f32, kind="ExternalInput")
    mk_d = nc.dram_tensor("mk", [rows, NP], f32, kind="ExternalInput")
    tj_d = nc.dram_tensor("tj", [ns, 3, T], f32, kind="ExternalInput")
    m4_d = nc.dram_tensor("m4", [RT, spt], f32, kind="ExternalInput")
    out_d = nc.dram_tensor("out", [ns, T, 3], f32, kind="ExternalOutput")

    with tile.TileContext(nc) as tc:
        with (
            tc.tile_pool(name="work", bufs=2) as wp,
            tc.tile_pool(name="fin", bufs=1) as fp,
            tc.tile_pool(name="ps", bufs=1, space="PSUM") as pp,
            tc.tile_pool(name="dram", bufs=1, space="DRAM") as dp,
        ):
            cost_s = dp.tile([rows], f32)
            oh_s = dp.tile([rows], f32)
            mask4 = fp.tile([RT, spt], f32)
            nc.sync.dma_start(out=mask4, in_=m4_d.ap())

            projs = []
            for k in range(ntiles):
                p = min(RT, rows - k * RT)
                r0 = k * RT

                rpt = wp.tile([p, 3, NP], f32, tag="rpt")
                nc.sync.dma_start(out=rpt, in_=rp_d.ap()[r0:r0 + p])
                mt = wp.tile([p, NP], f32, tag="mt")
                nc.sync.dma_start(out=mt, in_=mk_d.ap()[r0:r0 + p])
                # trajectory of each row's sample, broadcast to its 32 rows
                tpb = wp.tile([p, 3, T], f32, tag="tpb")
                nc.sync.dma_start(out=tpb, in_=bass.AP(
                    tensor=tj_d.ap().tensor, offset=k * spt * 3 * T,
                    ap=[[3 * T, spt], [0, NB2], [1, 3 * T]]))

                # --- segment data ---
                sv = wp.tile([p, 3, NSEG], f32, tag="sv")
                nc.vector.tensor_sub(out=sv, in0=rpt[:, :, 1:NP],
                                     in1=rpt[:, :, 0:NSEG])
                sm = wp.tile([p, NSEG], f32, tag="sm")
                nc.vector.tensor_mul(out=sm, in0=mt[:, 1:NP], in1=mt[:, 0:NSEG])
                sq3 = wp.tile([p, 3, NSEG], f32, tag="sq3")
                nc.vector.tensor_mul(out=sq3, in0=sv, in1=sv)
                sl2 = wp.tile([p, NSEG], f32, tag="sl2")
                nc.vector.tensor_reduce(out=sl2,
                                        in_=_view(sq3, [[1, NSEG], [NSEG, 3]]),
                                        axis=AX.X, op=AT.add)
                sl2m = wp.tile([p, NSEG], f32, tag="sl2m")
                nc.vector.tensor_mul(out=sl2m, in0=sl2, in1=sm)
                sl = wp.tile([p, NSEG], f32, tag="sl")
                eps2 = wp.tile([p, 1], f32, tag="eps2")
                nc.vector.memset(eps2, 1e-18)
                nc.scalar.activation(out=sl, in_=sl2m,
                                     func=mybir.ActivationFunctionType.Sqrt,
                                     bias=eps2)

                cum = wp.tile([p, NP], f32, tag="cum")
                zc = wp.tile([p, 1], f32, tag="zc")
                nc.vector.memset(zc, 0.0)
                nc.vector.memset(cum[:, 0:1], 0.0)
                nc.vector.tensor_tensor_scan(
                    out=cum[:, 1:NP], data0=sl, data1=_view(zc, [[0, NSEG]]),
                    initial=0.0, op0=AT.add, op1=AT.add)
                total = cum[:, NP - 1:NP]
                # negated cumsum for the hinge stream (on ACT, frees DVE)
                cumneg = wp.tile([p, NSEG], f32, tag="cumneg")
                nc.scalar.activation(out=cumneg, in_=cum[:, 0:NSEG],
                                     func=mybir.ActivationFunctionType.Copy,
                                     scale=-1.0)
                rsl = wp.tile([p, NSEG], f32, tag="rsl")
                rscr = wp.tile([p, NSEG], f32, tag="rscr")
                nc.vector.reciprocal_approx_accurate(out=rsl, in_=sl,
                                                     scratch=rscr)
                rslm = wp.tile([p, NSEG], f32, tag="rslm")
                nc.vector.tensor_mul(out=rslm, in0=rsl, in1=sm)
                wm = wp.tile([p, 3, NSEG], f32, tag="wm")
                nc.vector.tensor_mul(out=wm, in0=sv,
                                     in1=_view(rslm, [[0, 3], [1, NSEG]]))
                # dw[c,0] = wm[c,0]; dw[c,j] = wm[c,j]-wm[c,j-1]
                dw = wp.tile([p, 3, NSEG], f32, tag="dw")
                nc.vector.tensor_copy(out=_view(dw, [[NSEG, 3]]),
                                      in_=_view(wm, [[NSEG, 3]]))
                nc.vector.tensor_sub(
                    out=_view(dw, [[NSEG, 3], [1, NSEG - 1]], extra_off=1),
                    in0=_view(wm, [[NSEG, 3], [1, NSEG - 1]], extra_off=1),
                    in1=_view(wm, [[NSEG, 3], [1, NSEG - 1]]))

                # --- project p0 on all segments; entry_s via one-hot argmin ---
                tmp3 = wp.tile([p, 3, NSEG], f32, tag="tmp3")
                for c in range(3):
                    # (a_c - p0_c) * sv_c
                    nc.vector.scalar_tensor_tensor(
                        out=tmp3[:, c, :], in0=rpt[:, c, 0:NSEG],
                        scalar=tpb[:, c, 0:1], in1=sv[:, c, :],
                        op0=AT.subtract, op1=AT.mult)
                dotn = wp.tile([p, NSEG], f32, tag="dotn")
                nc.vector.tensor_reduce(out=dotn,
                                        in_=_view(tmp3, [[1, NSEG], [NSEG, 3]]),
                                        axis=AX.X, op=AT.add)
                svd = wp.tile([p, NSEG], f32, tag="svd")
                nc.vector.tensor_scalar(out=svd, in0=sl2, scalar1=1e-12,
                                        scalar2=None, op0=AT.max)
                rsvd = wp.tile([p, NSEG], f32, tag="rsvd")
                nc.vector.reciprocal_approx_accurate(out=rsvd, in_=svd,
                                                     scratch=rscr)
                t0 = wp.tile([p, NSEG], f32, tag="t0")
                nc.vector.tensor_mul(out=t0, in0=dotn, in1=rsvd)
                # t0 = min(max(-t0, 0), 1)
                nc.vector.tensor_scalar(out=t0, in0=t0, scalar1=-1.0,
                                        scalar2=0.0, op0=AT.mult, op1=AT.max)
                nc.vector.tensor_scalar(out=t0, in0=t0, scalar1=1.0,
                                        scalar2=None, op0=AT.min)
                s3 = wp.tile([p, 3, NSEG], f32, tag="s3")
                nc.vector.tensor_mul(out=s3, in0=sv,
                                     in1=_view(t0, [[0, 3], [1, NSEG]]))
                e3 = wp.tile([p, 3, NSEG], f32, tag="e3")
                for c in range(3):
                    # (a_c - p0_c) + t0*sv_c  (= q0_c - p0_c)
                    nc.vector.scalar_tensor_tensor(
                        out=e3[:, c, :], in0=rpt[:, c, 0:NSEG],
                        scalar=tpb[:, c, 0:1], in1=s3[:, c, :],
                        op0=AT.subtract, op1=AT.add)
                e3sq = wp.tile([p, 3, NSEG], f32, tag="e3sq")
                nc.vector.tensor_mul(out=e3sq, in0=e3, in1=e3)
                d2 = wp.tile([p, NSEG], f32, tag="d2")
                nc.vector.tensor_reduce(out=d2,
                                        in_=_view(e3sq, [[1, NSEG], [NSEG, 3]]),
                                        axis=AX.X, op=AT.add)
                d2m = wp.tile([p, NSEG], f32, tag="d2m")
                # d2m = d2 + (1-sm)*BIG  (sm is exactly 0/1)
                nc.vector.tensor_scalar(out=d2m, in0=sm, scalar1=1.0,
                                        scalar2=-BIG, op0=AT.subtract,
                                        op1=AT.mult)
                nc.vector.tensor_add(out=d2m, in0=d2m, in1=d2)
                dmin = wp.tile([p, 1], f32, tag="dmin")
                nc.vector.tensor_reduce(out=dmin, in_=d2m, axis=AX.X, op=AT.min)
                ohseg = wp.tile([p, NSEG], f32, tag="ohseg")
                nc.vector.tensor_scalar(out=ohseg, in0=d2m, scalar1=dmin,
                                        scalar2=None, op0=AT.is_equal)
                # keep only the FIRST hot via prefix-max diff (jnp.argmin ties)
                pmax = wp.tile([p, NSEG], f32, tag="pmax")
                nc.vector.tensor_tensor_scan(
                    out=pmax, data0=ohseg, data1=_view(zc, [[0, NSEG]]),
                    initial=0.0, op0=AT.max, op1=AT.add)
                nc.vector.tensor_copy(out=ohseg[:, 0:1], in_=pmax[:, 0:1])
                nc.vector.tensor_sub(out=ohseg[:, 1:NSEG], in0=pmax[:, 1:NSEG],
                                     in1=pmax[:, 0:NSEG - 1])
                es = wp.tile([p, NSEG], f32, tag="es")
                nc.vector.tensor_mul(out=es, in0=t0, in1=sl)
                nc.vector.tensor_add(out=es, in0=es, in1=cum[:, 0:NSEG])
                entry = wp.tile([p, 1], f32, tag="entry")
                junk0 = wp.tile([p, NSEG], f32, tag="junk0")
                nc.vector.scalar_tensor_tensor(
                    out=junk0, in0=ohseg, scalar=1.0, in1=es,
                    op0=AT.mult, op1=AT.mult, accum_out=entry)

                # --- base point rp[first valid segment] ---
                ohf = wp.tile([p, NSEG], f32, tag="ohf")
                nc.vector.tensor_copy(out=ohf[:, 0:1], in_=sm[:, 0:1])
                nc.vector.tensor_sub(out=ohf[:, 1:NSEG], in0=sm[:, 1:NSEG],
                                     in1=sm[:, 0:NSEG - 1])
                nc.vector.tensor_scalar(out=ohf, in0=ohf, scalar1=0.0,
                                        scalar2=None, op0=AT.max)
                base3 = wp.tile([p, 3], f32, tag="base3")
                for c in range(3):
                    nc.vector.scalar_tensor_tensor(
                        out=junk0, in0=ohf, scalar=1.0, in1=rpt[:, c, 0:NSEG],
                        op0=AT.mult, op1=AT.mult,
                        accum_out=base3[:, c:c + 1])

                # --- trajectory cumulative arc length + target_s ---
                td = wp.tile([p, 3, T - 1], f32, tag="td")
                nc.vector.tensor_sub(out=td, in0=tpb[:, :, 1:T],
                                     in1=tpb[:, :, 0:T - 1])
                td2 = wp.tile([p, 3, T - 1], f32, tag="td2")
                nc.vector.tensor_mul(out=td2, in0=td, in1=td)
                tl2 = wp.tile([p, T - 1], f32, tag="tl2")
                nc.vector.tensor_reduce(out=tl2,
                                        in_=_view(td2, [[1, T - 1], [T - 1, 3]]),
                                        axis=AX.X, op=AT.add)
                tl = wp.tile([p, T - 1], f32, tag="tl")
                nc.scalar.sqrt(out=tl, in_=tl2)
                tcum = wp.tile([p, T], f32, tag="tcum")
                nc.vector.memset(tcum[:, 0:1], 0.0)
                nc.vector.tensor_tensor_scan(
                    out=tcum[:, 1:T], data0=tl, data1=_view(zc, [[0, T - 1]]),
                    initial=0.0, op0=AT.add, op1=AT.add)
                target = wp.tile([p, T], f32, tag="target")
                nc.vector.scalar_tensor_tensor(
                    out=target, in0=tcum, scalar=entry,
                    in1=_view(total, [[0, T]]), op0=AT.add, op1=AT.min)
                nc.vector.tensor_scalar(out=target, in0=target, scalar1=0.0,
                                        scalar2=None, op0=AT.max)

                # --- main pass: fused hinge-scan, one DVE instr per t;
                # the last GP_T timesteps run on Pool (gpsimd) + ACT accum ---
                TD = T - GP_T
                E = wp.tile([p, 3, T], f32, tag="E")
                scrs = [wp.tile([p, 3 * NSEG], f32, name=f"scr{k}_{i}")
                        for i in range(3)]
                cn_b = _view(cumneg, [[0, 3], [1, NSEG]])
                if GP_T > 0:
                    Eg = wp.tile([p, 3, GP_T], f32, tag="Eg")
                    vts = [wp.tile([p, NSEG], f32, name=f"vt{k}_{i}")
                           for i in range(2)]
                    mcs = [wp.tile([p, NSEG], f32, name=f"mc{k}_{i}")
                           for i in range(6)]
                for t in range(T):
                    if t < TD:
                        scr = scrs[t % 3]
                        nc.vector._custom_dve(
                            HINGE, out=scr, in0=cn_b, in1=dw,
                            s0=target[:, t:t + 1], s1=0.0)
                        nc.scalar.copy(
                            out=_view(E, [[T, 3]], extra_off=t),
                            in_=_view(scr, [[NSEG, 3]], extra_off=NSEG - 1))
                    else:
                        i = t - TD
                        vt = vts[i % 2]
                        nc.gpsimd.tensor_scalar(
                            out=vt, in0=cumneg, scalar1=target[:, t:t + 1],
                            scalar2=0.0, op0=AT.add, op1=AT.max)
                        for c in range(3):
                            mc = mcs[(3 * i + c) % 6]
                            nc.gpsimd.tensor_mul(out=mc, in0=vt,
                                                 in1=dw[:, c, :])
                            nc.scalar.activation(
                                out=mc, in_=mc,
                                func=mybir.ActivationFunctionType.Copy,
                                accum_out=Eg[:, c, i:i + 1])

                # page-end diffs + base -> proj
                proj = wp.tile([p, 3, T], f32, name=f"proj{k}", bufs=1)
                nc.vector.tensor_scalar(out=proj[:, 0, 0:TD], in0=E[:, 0, 0:TD],
                                        scalar1=base3[:, 0:1], scalar2=None,
                                        op0=AT.add)
                for c in (1, 2):
                    nc.vector.scalar_tensor_tensor(
                        out=proj[:, c, 0:TD], in0=E[:, c, 0:TD],
                        scalar=base3[:, c:c + 1], in1=E[:, c - 1, 0:TD],
                        op0=AT.add, op1=AT.subtract)
                if GP_T > 0:
                    for c in range(3):
                        nc.vector.tensor_scalar(
                            out=proj[:, c, TD:T], in0=Eg[:, c, :],
                            scalar1=base3[:, c:c + 1], scalar2=None,
                            op0=AT.add)
                projs.append(proj)

                # --- cost ---
                df2 = wp.tile([p, 3, T], f32, tag="df2")
                nc.vector._custom_dve(DIFFSQ, out=df2, in0=proj, in1=tpb,
                                      s0=0.0, s1=0.0)
                dd = wp.tile([p, T], f32, tag="dd")
                nc.vector.tensor_reduce(out=dd,
                                        in_=_view(df2, [[1, T], [T, 3]]),
                                        axis=AX.X, op=AT.add)
                dist = wp.tile([p, T], f32, tag="dist")
                cost = wp.tile([p, 1], f32, tag="cost")
                nc.scalar.activation(out=dist, in_=dd,
                                     func=mybir.ActivationFunctionType.Sqrt,
                                     accum_out=cost)
                nc.sync.dma_start(
                    out=_dview(cost_s, [[1, p]], extra_off=r0), in_=cost)

            # --- per-sample argmin over branches ---
            costT = fp.tile([ns, NB2], f32)
            nc.sync.dma_start(out=costT,
                              in_=_dview(cost_s, [[NB2, ns], [1, NB2]]))
            cmin = fp.tile([ns, 1], f32)
            nc.vector.tensor_reduce(out=cmin, in_=costT, axis=AX.X, op=AT.min)
            oh8 = fp.tile([ns, NB2], f32)
            nc.vector.tensor_scalar(out=oh8, in0=costT, scalar1=cmin,
                                    scalar2=None, op0=AT.is_equal)
            zc8 = fp.tile([ns, 1], f32)
            nc.vector.memset(zc8, 0.0)
            pm8 = fp.tile([ns, NB2], f32)
            nc.vector.tensor_tensor_scan(
                out=pm8, data0=oh8, data1=_view(zc8, [[0, NB2]]),
                initial=0.0, op0=AT.max, op1=AT.add)
            nc.vector.tensor_copy(out=oh8[:, 0:1], in_=pm8[:, 0:1])
            nc.vector.tensor_sub(out=oh8[:, 1:NB2], in0=pm8[:, 1:NB2],
                                 in1=pm8[:, 0:NB2 - 1])
            nc.sync.dma_start(out=_dview(oh_s, [[NB2, ns], [1, NB2]]), in_=oh8)

            # --- best-branch gather via PE: block-diag one-hot matmul ---
            for k in range(ntiles):
                p = min(RT, rows - k * RT)
                ohcol = fp.tile([p, 1], f32, name=f"ohcol{k}")
                nc.sync.dma_start(out=ohcol,
                                  in_=_dview(oh_s, [[1, p]], extra_off=k * RT))
                ohbd = fp.tile([p, spt], f32, name=f"ohbd{k}")
                nc.vector.tensor_scalar(out=ohbd, in0=mask4, scalar1=ohcol,
                                        scalar2=None, op0=AT.mult)
                pj = pp.tile([spt, 3 * T], f32, name=f"pj{k}")
                nc.tensor.matmul(pj, ohbd, projs[k], start=True, stop=True)
                outt = fp.tile([spt, T, 3], f32, name=f"outt{k}")
                nc.scalar.copy(out=_view(outt, [[1, 3], [3, T]]),
                               in_=_view(pj, [[T, 3], [1, T]]))
                nc.sync.dma_start(out=out_d.ap()[k * spt:(k + 1) * spt],
                                  in_=outt)

    nc.compile()
    return nc


def marshal_inputs(selected_traj, road_points, road_mask):
    """Host-side layout marshaling (permutations/casts only): per-core input
    dicts with fwd+bwd branch rows and planar (xyz-major) layouts."""
    st = np.ascontiguousarray(selected_traj, dtype=np.float32)
    rp = np.ascontiguousarray(road_points, dtype=np.float32)
    rm = np.asarray(road_mask)

    rp_ext = np.concatenate([rp, rp[:, :, ::-1, :]], axis=1)        # [N,NB2,NP,3]
    rp_ext = np.ascontiguousarray(rp_ext.transpose(0, 1, 3, 2))     # [N,NB2,3,NP]
    mk_ext = np.concatenate([rm, rm[:, :, ::-1]], axis=1).astype(np.float32)
    tj = np.ascontiguousarray(st.transpose(0, 2, 1))                # [N,3,T]

    spt = RT // NB2
    m4 = np.zeros((RT, spt), dtype=np.float32)
    for s in range(spt):
        m4[s * NB2:(s + 1) * NB2, s] = 1.0

    in_maps = []
    for c in range(NCORES):
        s = slice(c * NS, (c + 1) * NS)
        in_maps.append({
            "rp": np.ascontiguousarray(rp_ext[s]).reshape(NS * NB2, 3, NP),
            "mk": np.ascontiguousarray(mk_ext[s]).reshape(NS * NB2, NP),
            "tj": np.ascontiguousarray(tj[s]),
            "m4": m4,
        })
    return in_maps


_NC = None


def kernel(selected_traj, road_points, road_mask):
    global _NC
    if _NC is None:
        _NC = build_nc()
    in_maps = marshal_inputs(selected_traj, road_points, road_mask)
    res = run_bass_kernel_spmd(_NC, in_maps, core_ids=list(range(NCORES)))
    out = np.concatenate([r["out"] for r in res.results], axis=0)
    return out.astype(np.float32)


# revision 15
# speedup vs baseline: 1.0165x; 1.0016x over previous
"""Trainium2 Bass kernel for the arc-projection problem.

Full-input contract: kernel(**inputs) takes the unsharded numpy inputs and
returns the full output. Internally shards the batch N=64 across 8 cores
(pure data parallel), runs one SPMD Bass kernel, and gathers.

Algorithm (matches reference._arc_projection, reformulated gather-free):
  For each (sample, branch, direction) row:
    - segment vectors sv, masked lengths sl, cumsum cum, masked unit
      tangents wm = sv/sl*mask, and their first differences dw.
    - project trajectory point 0 on all segments -> entry_s (one-hot argmin)
    - target_s[t] = clip(entry_s + traj_cum[t], 0, total)
    - proj_c(s) = base_c + sum_j dw_c[j] * relu(s - cum_j)   (telescoped
      hinge; equals the reference's searchsorted+lerp for prefix/suffix
      masks). Computed by a custom fused DVE op: one instruction per t
      producing running sums whose page-ends give all 3 channels.
    - cost = sum_t |pos_t - proj_t|; per-sample argmin over 32 rows via
      one-hot; best-branch gather via a PE matmul with a block-diagonal
      one-hot stationary (no DRAM round-trip).
"""

import sys

import numpy as np

try:
    import concourse.bass as bass
except ImportError:  # pragma: no cover - container without PYTHONPATH set
    sys.path.insert(0, "/opt/trn_rl_repo")
    import concourse.bass as bass

import concourse.tile as tile
from concourse import bacc, mybir
from concourse.bass_utils import run_bass_kernel_spmd

import concourse.dve_ops as _dmod
from concourse.dve_spec import (
    Spec, Src0, Src1, C0, relu, scan,
    AluOp as _AluOp, lower as _dve_lower, _has_src1,
)
from concourse.dve_uop import DveOpSpec as _DveOpSpec

f32 = mybir.dt.float32
AT = mybir.AluOpType
AX = mybir.AxisListType

N, T, NB, NP = 64, 128, 16, 256
NCORES = 8
NS = N // NCORES          # samples per core
NB2 = 2 * NB              # fwd + bwd branches
NSEG = NP - 1
NS2 = NP                  # padded segment count (256) for the hinge stream
BIG = 1.0e30
RT = 128                  # rows per partition-tile
GP_T = 0                  # Pool offload disabled: SBUF-port contention with DVE


def _register_hinge_op():
    """out[p,k] = cumsum_k(relu(in0[p,k] + s0[p]) * in1[p,k]) as one DVE op."""
    name = "HINGE_SCAN_ANT"
    for o in _dmod.OPS:
        if o.name == name:
            return o

    def _ref(in0, in1, s0, s1, imm2):
        x = (np.maximum(in0.astype(np.float32) + s0, 0.0) * in1).astype(np.float32)
        xf = x.reshape(x.shape[0], -1)
        return np.cumsum(xf, axis=-1).astype(np.float32).reshape(x.shape)

    spec = Spec(body=scan(_AluOp.ADD, relu(Src0 + C0) * Src1), reference=_ref)
    op = _dmod.DveOp(name, spec, subdim=False, uops_sha={})
    _dmod.OPS.append(op)
    _dmod._SUB_OPCODE_FOR_NAME[name] = _dmod._CUSTOM_DVE_ROW_BASE + len(_dmod.OPS) - 1
    for ver in ("v3", "v4"):
        _dmod._COMPILE_CACHE[(name, ver)] = _DveOpSpec(
            name=name,
            opcode=_dmod._SUB_OPCODE_FOR_NAME[name],
            uops=_dve_lower(spec, ver=ver),
            rd1_en=_has_src1(spec),
        )
    return op


HINGE = _register_hinge_op()


def _register_diffsq_op():
    """out[p,k] = (in0[p,k] - in1[p,k])**2 as one DVE op."""
    name = "DIFFSQ_ANT"
    for o in _dmod.OPS:
        if o.name == name:
            return o
    from concourse.dve_spec import sq as _sq
    spec = Spec(
        body=_sq(Src0 - Src1),
        reference=lambda in0, in1, s0, s1, imm2:
            ((in0.astype(np.float32) - in1) ** 2).astype(np.float32),
    )
    op = _dmod.DveOp(name, spec, subdim=False, uops_sha={})
    _dmod.OPS.append(op)
    _dmod._SUB_OPCODE_FOR_NAME[name] = _dmod._CUSTOM_DVE_ROW_BASE + len(_dmod.OPS) - 1
    for ver in ("v3", "v4"):
        _dmod._COMPILE_CACHE[(name, ver)] = _DveOpSpec(
            name=name,
            opcode=_dmod._SUB_OPCODE_FOR_NAME[name],
            uops=_dve_lower(spec, ver=ver),
            rd1_en=_has_src1(spec),
        )
    return op


DIFFSQ = _register_diffsq_op()


def _view(t, ap_dims, extra_off=0):
    """Strided view of a tile/AP: ap_dims are [step, count] free dims after
    the partition dim (kept from t)."""
    return bass.AP(tensor=t.tensor, offset=t.offset + extra_off,
                   ap=[t.ap[0]] + ap_dims)


def _dview(t, ap_dims, extra_off=0):
    """Raw view of a DRAM tile: ap_dims replace all dims."""
    return bass.AP(tensor=t.tensor, offset=t.offset + extra_off, ap=ap_dims)


def build_nc(ns=NS, enable_asserts=False):
    rows = ns * NB2
    ntiles = (rows + RT - 1) // RT
    spt = RT // NB2  # samples per tile

    nc = bacc.Bacc("TRN2", target_bir_lowering=False, debug=False,
                   enable_asserts=enable_asserts, num_devices=NCORES)

    rp_d = nc.dram_tensor("rp", [rows, 3, NP], f32, kind="ExternalInput")
    mk_d = nc.dram_tensor("mk", [rows, NP], f32, kind="ExternalInput")
    tj_d = nc.dram_tensor("tj", [ns, 3, T], f32, kind="ExternalInput")
    m4_d = nc.dram_tensor("m4", [RT, spt], f32, kind="ExternalInput")
    out_d = nc.dram_tensor("out", [ns, T, 3], f32, kind="ExternalOutput")

    with tile.TileContext(nc) as tc:
        with (
            tc.tile_pool(name="work", bufs=2) as wp,
            tc.tile_pool(name="fin", bufs=1) as fp,
            tc.tile_pool(name="ps", bufs=1, space="PSUM") as pp,
            tc.tile_pool(name="dram", bufs=1, space="DRAM") as dp,
        ):
            cost_s = dp.tile([rows], f32)
            oh_s = dp.tile([rows], f32)
            mask4 = fp.tile([RT, spt], f32)
            nc.sync.dma_start(out=mask4, in_=m4_d.ap())

            projs = []
            done_endgame = set()

            def emit_endgame(k):
                if k < 0 or k >= len(projs) or k in done_endgame:
                    return
                done_endgame.add(k)
                pk = min(RT, rows - k * RT)
                rk = k * RT
                costT = fp.tile([spt, NB2], f32, name=f"costT{k}")
                nc.sync.dma_start(
                    out=costT,
                    in_=_dview(cost_s, [[NB2, spt], [1, NB2]], extra_off=rk))
                cmin = fp.tile([spt, 1], f32, name=f"cmin{k}")
                nc.vector.tensor_reduce(out=cmin, in_=costT, axis=AX.X,
                                        op=AT.min)
                oh8 = fp.tile([spt, NB2], f32, name=f"oh8{k}")
                nc.vector.tensor_scalar(out=oh8, in0=costT, scalar1=cmin,
                                        scalar2=None, op0=AT.is_equal)
                zc8 = fp.tile([spt, 1], f32, name=f"zc8{k}")
                nc.vector.memset(zc8, 0.0)
                pm8 = fp.tile([spt, NB2], f32, name=f"pm8{k}")
                nc.vector.tensor_tensor_scan(
                    out=pm8, data0=oh8, data1=_view(zc8, [[0, NB2]]),
                    initial=0.0, op0=AT.max, op1=AT.add)
                nc.vector.tensor_copy(out=oh8[:, 0:1], in_=pm8[:, 0:1])
                nc.vector.tensor_sub(out=oh8[:, 1:NB2], in0=pm8[:, 1:NB2],
                                     in1=pm8[:, 0:NB2 - 1])
                nc.sync.dma_start(
                    out=_dview(oh_s, [[NB2, spt], [1, NB2]], extra_off=rk),
                    in_=oh8)
                ohcol = fp.tile([pk, 1], f32, name=f"ohcol{k}")
                nc.sync.dma_start(out=ohcol,
                                  in_=_dview(oh_s, [[1, pk]], extra_off=rk))
                ohbd = fp.tile([pk, spt], f32, name=f"ohbd{k}")
                nc.vector.tensor_scalar(out=ohbd, in0=mask4, scalar1=ohcol,
                                        scalar2=None, op0=AT.mult)
                pj = pp.tile([spt, 3 * T], f32, name=f"pj{k}")
                nc.tensor.matmul(pj, ohbd, projs[k], start=True, stop=True)
                outt = fp.tile([spt, T, 3], f32, name=f"outt{k}")
                nc.scalar.copy(out=_view(outt, [[1, 3], [3, T]]),
                               in_=_view(pj, [[T, 3], [1, T]]))
                nc.sync.dma_start(out=out_d.ap()[k * spt:(k + 1) * spt],
                                  in_=outt)
            for k in range(ntiles):
                p = min(RT, rows - k * RT)
                r0 = k * RT

                rpt = wp.tile([p, 3, NP], f32, tag="rpt")
                nc.sync.dma_start(out=rpt, in_=rp_d.ap()[r0:r0 + p])
                mt = wp.tile([p, NP], f32, tag="mt")
                nc.sync.dma_start(out=mt, in_=mk_d.ap()[r0:r0 + p])
                # trajectory of each row's sample, broadcast to its 32 rows
                tpb = wp.tile([p, 3, T], f32, tag="tpb")
                nc.sync.dma_start(out=tpb, in_=bass.AP(
                    tensor=tj_d.ap().tensor, offset=k * spt * 3 * T,
                    ap=[[3 * T, spt], [0, NB2], [1, 3 * T]]))

                # --- segment data ---
                sv = wp.tile([p, 3, NSEG], f32, tag="sv")
                nc.vector.tensor_sub(out=sv, in0=rpt[:, :, 1:NP],
                                     in1=rpt[:, :, 0:NSEG])
                sm = wp.tile([p, NSEG], f32, tag="sm")
                nc.vector.tensor_mul(out=sm, in0=mt[:, 1:NP], in1=mt[:, 0:NSEG])
                sq3 = wp.tile([p, 3, NSEG], f32, tag="sq3")
                nc.vector.tensor_mul(out=sq3, in0=sv, in1=sv)
                sl2 = wp.tile([p, NSEG], f32, tag="sl2")
                nc.vector.tensor_reduce(out=sl2,
                                        in_=_view(sq3, [[1, NSEG], [NSEG, 3]]),
                                        axis=AX.X, op=AT.add)
                sl2m = wp.tile([p, NSEG], f32, tag="sl2m")
                nc.vector.tensor_mul(out=sl2m, in0=sl2, in1=sm)
                sl = wp.tile([p, NSEG], f32, tag="sl")
                eps2 = wp.tile([p, 1], f32, tag="eps2")
                nc.vector.memset(eps2, 1e-18)
                nc.scalar.activation(out=sl, in_=sl2m,
                                     func=mybir.ActivationFunctionType.Sqrt,
                                     bias=eps2)

                zc = wp.tile([p, 1], f32, tag="zc")
                nc.vector.memset(zc, 0.0)
                rscr = wp.tile([p, NSEG], f32, tag="rscr")
                # --- project p0 on all segments; entry_s via one-hot argmin ---
                tmp3 = wp.tile([p, 3, NSEG], f32, tag="tmp3")
                for c in range(3):
                    # (a_c - p0_c) * sv_c
                    nc.vector.scalar_tensor_tensor(
                        out=tmp3[:, c, :], in0=rpt[:, c, 0:NSEG],
                        scalar=tpb[:, c, 0:1], in1=sv[:, c, :],
                        op0=AT.subtract, op1=AT.mult)
                dotn = wp.tile([p, NSEG], f32, tag="dotn")
                nc.vector.tensor_reduce(out=dotn,
                                        in_=_view(tmp3, [[1, NSEG], [NSEG, 3]]),
                                        axis=AX.X, op=AT.add)
                svd = wp.tile([p, NSEG], f32, tag="svd")
                nc.vector.tensor_scalar(out=svd, in0=sl2, scalar1=1e-12,
                                        scalar2=None, op0=AT.max)
                rsvd = wp.tile([p, NSEG], f32, tag="rsvd")
                nc.vector.reciprocal_approx_accurate(out=rsvd, in_=svd,
                                                     scratch=rscr)
                t0 = wp.tile([p, NSEG], f32, tag="t0")
                nc.vector.tensor_mul(out=t0, in0=dotn, in1=rsvd)
                # t0 = min(max(-t0, 0), 1)
                nc.vector.tensor_scalar(out=t0, in0=t0, scalar1=-1.0,
                                        scalar2=0.0, op0=AT.mult, op1=AT.max)
                nc.vector.tensor_scalar(out=t0, in0=t0, scalar1=1.0,
                                        scalar2=None, op0=AT.min)
                s3 = wp.tile([p, 3, NSEG], f32, tag="s3")
                nc.vector.tensor_mul(out=s3, in0=sv,
                                     in1=_view(t0, [[0, 3], [1, NSEG]]))
                e3 = wp.tile([p, 3, NSEG], f32, tag="e3")
                for c in range(3):
                    # (a_c - p0_c) + t0*sv_c  (= q0_c - p0_c)
                    nc.vector.scalar_tensor_tensor(
                        out=e3[:, c, :], in0=rpt[:, c, 0:NSEG],
                        scalar=tpb[:, c, 0:1], in1=s3[:, c, :],
                        op0=AT.subtract, op1=AT.add)
                e3sq = wp.tile([p, 3, NSEG], f32, tag="e3sq")
                nc.vector.tensor_mul(out=e3sq, in0=e3, in1=e3)
                d2 = wp.tile([p, NSEG], f32, tag="d2")
                nc.vector.tensor_reduce(out=d2,
                                        in_=_view(e3sq, [[1, NSEG], [NSEG, 3]]),
                                        axis=AX.X, op=AT.add)
                d2m = wp.tile([p, NSEG], f32, tag="d2m")
                # d2m = d2 + (1-sm)*BIG  (sm is exactly 0/1)
                nc.vector.tensor_scalar(out=d2m, in0=sm, scalar1=1.0,
                                        scalar2=-BIG, op0=AT.subtract,
                                        op1=AT.mult)
                nc.vector.tensor_add(out=d2m, in0=d2m, in1=d2)
                dmin = wp.tile([p, 1], f32, tag="dmin")
                nc.vector.tensor_reduce(out=dmin, in_=d2m, axis=AX.X, op=AT.min)
                ohseg = wp.tile([p, NSEG], f32, tag="ohseg")
                nc.vector.tensor_scalar(out=ohseg, in0=d2m, scalar1=dmin,
                                        scalar2=None, op0=AT.is_equal)
                # keep only the FIRST hot via prefix-max diff (jnp.argmin ties)
                pmax = wp.tile([p, NSEG], f32, tag="pmax")
                nc.vector.tensor_tensor_scan(
                    out=pmax, data0=ohseg, data1=_view(zc, [[0, NSEG]]),
                    initial=0.0, op0=AT.max, op1=AT.add)
                nc.vector.tensor_copy(out=ohseg[:, 0:1], in_=pmax[:, 0:1])
                nc.vector.tensor_sub(out=ohseg[:, 1:NSEG], in0=pmax[:, 1:NSEG],
                                     in1=pmax[:, 0:NSEG - 1])
                cum = wp.tile([p, NP], f32, tag="cum")
                nc.vector.memset(cum[:, 0:1], 0.0)
                nc.vector.tensor_tensor_scan(
                    out=cum[:, 1:NP], data0=sl, data1=_view(zc, [[0, NSEG]]),
                    initial=0.0, op0=AT.add, op1=AT.add)
                total = cum[:, NP - 1:NP]
                # negated cumsum for the hinge stream (on ACT, frees DVE)
                cumneg = wp.tile([p, NSEG], f32, tag="cumneg")
                nc.scalar.activation(out=cumneg, in_=cum[:, 0:NSEG],
                                     func=mybir.ActivationFunctionType.Copy,
                                     scale=-1.0)
                rsl = wp.tile([p, NSEG], f32, tag="rsl")
                nc.vector.reciprocal_approx_accurate(out=rsl, in_=sl,
                                                     scratch=rscr)
                rslm = wp.tile([p, NSEG], f32, tag="rslm")
                nc.vector.tensor_mul(out=rslm, in0=rsl, in1=sm)
                wm = wp.tile([p, 3, NSEG], f32, tag="wm")
                nc.vector.tensor_mul(out=wm, in0=sv,
                                     in1=_view(rslm, [[0, 3], [1, NSEG]]))
                # dw[c,0] = wm[c,0]; dw[c,j] = wm[c,j]-wm[c,j-1]
                dw = wp.tile([p, 3, NSEG], f32, tag="dw")
                nc.scalar.copy(out=_view(dw, [[NSEG, 3]]),
                               in_=_view(wm, [[NSEG, 3]]))
                nc.vector.tensor_sub(
                    out=_view(dw, [[NSEG, 3], [1, NSEG - 1]], extra_off=1),
                    in0=_view(wm, [[NSEG, 3], [1, NSEG - 1]], extra_off=1),
                    in1=_view(wm, [[NSEG, 3], [1, NSEG - 1]]))

                es = wp.tile([p, NSEG], f32, tag="es")
                nc.vector.tensor_mul(out=es, in0=t0, in1=sl)
                nc.vector.tensor_add(out=es, in0=es, in1=cum[:, 0:NSEG])
                entry = wp.tile([p, 1], f32, tag="entry")
                junk0 = wp.tile([p, NSEG], f32, tag="junk0")
                nc.vector.scalar_tensor_tensor(
                    out=junk0, in0=ohseg, scalar=1.0, in1=es,
                    op0=AT.mult, op1=AT.mult, accum_out=entry)

                # --- base point rp[first valid segment] ---
                ohf = wp.tile([p, NSEG], f32, tag="ohf")
                nc.vector.tensor_copy(out=ohf[:, 0:1], in_=sm[:, 0:1])
                nc.vector.tensor_sub(out=ohf[:, 1:NSEG], in0=sm[:, 1:NSEG],
                                     in1=sm[:, 0:NSEG - 1])
                nc.vector.tensor_scalar(out=ohf, in0=ohf, scalar1=0.0,
                                        scalar2=None, op0=AT.max)
                base3 = wp.tile([p, 3], f32, tag="base3")
                for c in range(3):
                    nc.vector.scalar_tensor_tensor(
                        out=junk0, in0=ohf, scalar=1.0, in1=rpt[:, c, 0:NSEG],
                        op0=AT.mult, op1=AT.mult,
                        accum_out=base3[:, c:c + 1])

                # --- trajectory cumulative arc length + target_s ---
                td = wp.tile([p, 3, T - 1], f32, tag="td")
                nc.vector.tensor_sub(out=td, in0=tpb[:, :, 1:T],
                                     in1=tpb[:, :, 0:T - 1])
                td2 = wp.tile([p, 3, T - 1], f32, tag="td2")
                nc.vector.tensor_mul(out=td2, in0=td, in1=td)
                tl2 = wp.tile([p, T - 1], f32, tag="tl2")
                nc.vector.tensor_reduce(out=tl2,
                                        in_=_view(td2, [[1, T - 1], [T - 1, 3]]),
                                        axis=AX.X, op=AT.add)
                tl = wp.tile([p, T - 1], f32, tag="tl")
                nc.scalar.sqrt(out=tl, in_=tl2)
                tcum = wp.tile([p, T], f32, tag="tcum")
                nc.vector.memset(tcum[:, 0:1], 0.0)
                nc.vector.tensor_tensor_scan(
                    out=tcum[:, 1:T], data0=tl, data1=_view(zc, [[0, T - 1]]),
                    initial=0.0, op0=AT.add, op1=AT.add)
                target = wp.tile([p, T], f32, tag="target")
                nc.vector.scalar_tensor_tensor(
                    out=target, in0=tcum, scalar=entry,
                    in1=_view(total, [[0, T]]), op0=AT.add, op1=AT.min)
                nc.vector.tensor_scalar(out=target, in0=target, scalar1=0.0,
                                        scalar2=None, op0=AT.max)

                # --- main pass: fused hinge-scan, one DVE instr per t;
                # the last GP_T timesteps run on Pool (gpsimd) + ACT accum ---
                TD = T - GP_T
                E = wp.tile([p, 3, T], f32, tag="E")
                scrs = [wp.tile([p, 3 * NSEG], f32, name=f"scr{k}_{i}")
                        for i in range(3)]
                cn_b = _view(cumneg, [[0, 3], [1, NSEG]])
                emit_endgame(k - 1)
                if GP_T > 0:
                    Eg = wp.tile([p, 3, GP_T], f32, tag="Eg")
                    vts = [wp.tile([p, NSEG], f32, name=f"vt{k}_{i}")
                           for i in range(2)]
                    mcs = [wp.tile([p, NSEG], f32, name=f"mc{k}_{i}")
                           for i in range(6)]
                for t in range(T):
                    if t < TD:
                        scr = scrs[t % 3]
                        nc.vector._custom_dve(
                            HINGE, out=scr, in0=cn_b, in1=dw,
                            s0=target[:, t:t + 1], s1=0.0)
                        nc.scalar.copy(
                            out=_view(E, [[T, 3]], extra_off=t),
                            in_=_view(scr, [[NSEG, 3]], extra_off=NSEG - 1))
                    else:
                        i = t - TD
                        vt = vts[i % 2]
                        nc.gpsimd.tensor_scalar(
                            out=vt, in0=cumneg, scalar1=target[:, t:t + 1],
                            scalar2=0.0, op0=AT.add, op1=AT.max)
                        for c in range(3):
                            mc = mcs[(3 * i + c) % 6]
                            nc.gpsimd.tensor_mul(out=mc, in0=vt,
                                                 in1=dw[:, c, :])
                            nc.scalar.activation(
                                out=mc, in_=mc,
                                func=mybir.ActivationFunctionType.Copy,
                                accum_out=Eg[:, c, i:i + 1])

                # page-end diffs + base -> proj
                proj = wp.tile([p, 3, T], f32, name=f"proj{k}", bufs=1)
                nc.vector.tensor_scalar(out=proj[:, 0, 0:TD], in0=E[:, 0, 0:TD],
                                        scalar1=base3[:, 0:1], scalar2=None,
                                        op0=AT.add)
                for c in (1, 2):
                    nc.vector.scalar_tensor_tensor(
                        out=proj[:, c, 0:TD], in0=E[:, c, 0:TD],
                        scalar=base3[:, c:c + 1], in1=E[:, c - 1, 0:TD],
                        op0=AT.add, op1=AT.subtract)
                if GP_T > 0:
                    for c in range(3):
                        nc.vector.tensor_scalar(
                            out=proj[:, c, TD:T], in0=Eg[:, c, :],
                            scalar1=base3[:, c:c + 1], scalar2=None,
                            op0=AT.add)
                projs.append(proj)

                # --- cost ---
                df2 = wp.tile([p, 3, T], f32, tag="df2")
                nc.vector._custom_dve(DIFFSQ, out=df2, in0=proj, in1=tpb,
                                      s0=0.0, s1=0.0)
                dd = wp.tile([p, T], f32, tag="dd")
                nc.vector.tensor_reduce(out=dd,
                                        in_=_view(df2, [[1, T], [T, 3]]),
                                        axis=AX.X, op=AT.add)
                dist = wp.tile([p, T], f32, tag="dist")
                cost = wp.tile([p, 1], f32, tag="cost")
                nc.scalar.activation(out=dist, in_=dd,
                                     func=mybir.ActivationFunctionType.Sqrt,
                                     accum_out=cost)
                nc.sync.dma_start(
                    out=_dview(cost_s, [[1, p]], extra_off=r0), in_=cost)

            for k in range(ntiles):
                emit_endgame(k)

    nc.compile()
    return nc


def marshal_inputs(selected_traj, road_points, road_mask):
    """Host-side layout marshaling (permutations/casts only): per-core input
    dicts with fwd+bwd branch rows and planar (xyz-major) layouts."""
    st = np.ascontiguousarray(selected_traj, dtype=np.float32)
    rp = np.ascontiguousarray(road_points, dtype=np.float32)
    rm = np.asarray(road_mask)

    rp_ext = np.concatenate([rp, rp[:, :, ::-1, :]], axis=1)        # [N,NB2,NP,3]
    rp_ext = np.ascontiguousarray(rp_ext.transpose(0, 1, 3, 2))     # [N,NB2,3,NP]
    mk_ext = np.concatenate([rm, rm[:, :, ::-1]], axis=1).astype(np.float32)
    tj = np.ascontiguousarray(st.transpose(0, 2, 1))                # [N,3,T]

    spt = RT // NB2
    m4 = np.zeros((RT, spt), dtype=np.float32)
    for s in range(spt):
        m4[s * NB2:(s + 1) * NB2, s] = 1.0

    in_maps = []
    for c in range(NCORES):
        s = slice(c * NS, (c + 1) * NS)
        in_maps.append({
            "rp": np.ascontiguousarray(rp_ext[s]).reshape(NS * NB2, 3, NP),
            "mk": np.ascontiguousarray(mk_ext[s]).reshape(NS * NB2, NP),
            "tj": np.ascontiguousarray(tj[s]),
            "m4": m4,
        })
    return in_maps


_NC = None


def kernel(selected_traj, road_points, road_mask):
    global _NC
    if _NC is None:
        _NC = build_nc()
    in_maps = marshal_inputs(selected_traj, road_points, road_mask)
    res = run_bass_kernel_spmd(_NC, in_maps, core_ids=list(range(NCORES)))
    out = np.concatenate([r["out"] for r in res.results], axis=0)
    return out.astype(np.float32)


# revision 16
# speedup vs baseline: 1.0582x; 1.0411x over previous
"""Trainium2 Bass kernel for the arc-projection problem.

Full-input contract: kernel(**inputs) takes the unsharded numpy inputs and
returns the full output. Internally shards the batch N=64 across 8 cores
(pure data parallel), runs one SPMD Bass kernel, and gathers.

Algorithm (matches reference._arc_projection, reformulated gather-free):
  For each (sample, branch, direction) row:
    - segment vectors sv, masked lengths sl, cumsum cum, masked unit
      tangents wm = sv/sl*mask, and their first differences dw.
    - project trajectory point 0 on all segments -> entry_s (one-hot argmin)
    - target_s[t] = clip(entry_s + traj_cum[t], 0, total)
    - proj_c(s) = base_c + sum_j dw_c[j] * relu(s - cum_j)   (telescoped
      hinge; equals the reference's searchsorted+lerp for prefix/suffix
      masks). Computed by a custom fused DVE op: one instruction per t
      producing running sums whose page-ends give all 3 channels.
    - cost = sum_t |pos_t - proj_t|; per-sample argmin over 32 rows via
      one-hot; best-branch gather via a PE matmul with a block-diagonal
      one-hot stationary (no DRAM round-trip).
"""

import sys

import numpy as np

try:
    import concourse.bass as bass
except ImportError:  # pragma: no cover - container without PYTHONPATH set
    sys.path.insert(0, "/opt/trn_rl_repo")
    import concourse.bass as bass

import concourse.tile as tile
from concourse import bacc, mybir
from concourse.bass_utils import run_bass_kernel_spmd

import concourse.dve_ops as _dmod
from concourse.dve_spec import (
    Spec, Src0, Src1, C0, relu, scan,
    AluOp as _AluOp, lower as _dve_lower, _has_src1,
)
from concourse.dve_uop import DveOpSpec as _DveOpSpec

f32 = mybir.dt.float32
AT = mybir.AluOpType
AX = mybir.AxisListType

N, T, NB, NP = 64, 128, 16, 256
NCORES = 8
NS = N // NCORES          # samples per core
NB2 = 2 * NB              # fwd + bwd branches
NSEG = NP - 1
NS2 = NP                  # padded segment count (256) for the hinge stream
BIG = 1.0e30
RT = 128                  # rows per partition-tile
GP_T = 0                  # Pool offload disabled: SBUF-port contention with DVE


def _register_hinge_op():
    """out[p,k] = cumsum_k(relu(in0[p,k] + s0[p]) * in1[p,k]) as one DVE op."""
    name = "HINGE_SCAN_ANT"
    for o in _dmod.OPS:
        if o.name == name:
            return o

    def _ref(in0, in1, s0, s1, imm2):
        x = (np.maximum(in0.astype(np.float32) + s0, 0.0) * in1).astype(np.float32)
        xf = x.reshape(x.shape[0], -1)
        return np.cumsum(xf, axis=-1).astype(np.float32).reshape(x.shape)

    spec = Spec(body=scan(_AluOp.ADD, relu(Src0 + C0) * Src1), reference=_ref)
    op = _dmod.DveOp(name, spec, subdim=False, uops_sha={})
    _dmod.OPS.append(op)
    _dmod._SUB_OPCODE_FOR_NAME[name] = _dmod._CUSTOM_DVE_ROW_BASE + len(_dmod.OPS) - 1
    for ver in ("v3", "v4"):
        _dmod._COMPILE_CACHE[(name, ver)] = _DveOpSpec(
            name=name,
            opcode=_dmod._SUB_OPCODE_FOR_NAME[name],
            uops=_dve_lower(spec, ver=ver),
            rd1_en=_has_src1(spec),
        )
    return op


HINGE = _register_hinge_op()


def _register_diffsq_op():
    """out[p,k] = (in0[p,k] - in1[p,k])**2 as one DVE op."""
    name = "DIFFSQ_ANT"
    for o in _dmod.OPS:
        if o.name == name:
            return o
    from concourse.dve_spec import sq as _sq
    spec = Spec(
        body=_sq(Src0 - Src1),
        reference=lambda in0, in1, s0, s1, imm2:
            ((in0.astype(np.float32) - in1) ** 2).astype(np.float32),
    )
    op = _dmod.DveOp(name, spec, subdim=False, uops_sha={})
    _dmod.OPS.append(op)
    _dmod._SUB_OPCODE_FOR_NAME[name] = _dmod._CUSTOM_DVE_ROW_BASE + len(_dmod.OPS) - 1
    for ver in ("v3", "v4"):
        _dmod._COMPILE_CACHE[(name, ver)] = _DveOpSpec(
            name=name,
            opcode=_dmod._SUB_OPCODE_FOR_NAME[name],
            uops=_dve_lower(spec, ver=ver),
            rd1_en=_has_src1(spec),
        )
    return op


DIFFSQ = _register_diffsq_op()


def _view(t, ap_dims, extra_off=0):
    """Strided view of a tile/AP: ap_dims are [step, count] free dims after
    the partition dim (kept from t)."""
    return bass.AP(tensor=t.tensor, offset=t.offset + extra_off,
                   ap=[t.ap[0]] + ap_dims)


def _dview(t, ap_dims, extra_off=0):
    """Raw view of a DRAM tile: ap_dims replace all dims."""
    return bass.AP(tensor=t.tensor, offset=t.offset + extra_off, ap=ap_dims)


def build_nc(ns=NS, enable_asserts=False):
    rows = ns * NB2
    ntiles = (rows + RT - 1) // RT
    spt = RT // NB2  # samples per tile

    nc = bacc.Bacc("TRN2", target_bir_lowering=False, debug=False,
                   enable_asserts=enable_asserts, num_devices=NCORES)

    rp_d = nc.dram_tensor("rp", [rows, 3, NP], f32, kind="ExternalInput")
    mk_d = nc.dram_tensor("mk", [rows, NP], f32, kind="ExternalInput")
    tj_d = nc.dram_tensor("tj", [ns, 3, T], f32, kind="ExternalInput")
    m4_d = nc.dram_tensor("m4", [RT, spt], f32, kind="ExternalInput")
    m4t_d = nc.dram_tensor("m4t", [RT // NB2, RT], f32, kind="ExternalInput")
    mb_d = nc.dram_tensor("mb", [RT, NB2], f32, kind="ExternalInput")
    out_d = nc.dram_tensor("out", [ns, T, 3], f32, kind="ExternalOutput")

    with tile.TileContext(nc) as tc:
        with (
            tc.tile_pool(name="work", bufs=2) as wp,
            tc.tile_pool(name="fin", bufs=1) as fp,
            tc.tile_pool(name="ps", bufs=1, space="PSUM") as pp,
            tc.tile_pool(name="dram", bufs=1, space="DRAM") as dp,
        ):
            mask4 = fp.tile([RT, spt], f32)
            nc.sync.dma_start(out=mask4, in_=m4_d.ap())
            mask4t = fp.tile([spt, RT], f32)
            nc.sync.dma_start(out=mask4t, in_=m4t_d.ap())
            mb32 = fp.tile([RT, NB2], f32)
            nc.sync.dma_start(out=mb32, in_=mb_d.ap())

            projs = []
            costs = []
            done_endgame = set()

            def emit_endgame(k):
                if k < 0 or k >= len(projs) or k in done_endgame:
                    return
                done_endgame.add(k)
                pk = min(RT, rows - k * RT)
                rk = k * RT
                # costT = mask4^T @ (cost * mb32): [4, 32] sample-major, on-chip
                costm = fp.tile([pk, NB2], f32, name=f"costm{k}")
                nc.vector.tensor_scalar(out=costm, in0=mb32,
                                        scalar1=costs[k], scalar2=None,
                                        op0=AT.mult)
                psT = pp.tile([spt, NB2], f32, name=f"psT{k}")
                nc.tensor.matmul(psT, mask4, costm, start=True, stop=True)
                costT = fp.tile([spt, NB2], f32, name=f"costT{k}")
                nc.scalar.copy(out=costT, in_=psT)
                cmin = fp.tile([spt, 1], f32, name=f"cmin{k}")
                nc.vector.tensor_reduce(out=cmin, in_=costT, axis=AX.X,
                                        op=AT.min)
                oh8 = fp.tile([spt, NB2], f32, name=f"oh8{k}")
                nc.vector.tensor_scalar(out=oh8, in0=costT, scalar1=cmin,
                                        scalar2=None, op0=AT.is_equal)
                zc8 = fp.tile([spt, 1], f32, name=f"zc8{k}")
                nc.vector.memset(zc8, 0.0)
                pm8 = fp.tile([spt, NB2], f32, name=f"pm8{k}")
                nc.vector.tensor_tensor_scan(
                    out=pm8, data0=oh8, data1=_view(zc8, [[0, NB2]]),
                    initial=0.0, op0=AT.max, op1=AT.add)
                nc.vector.tensor_copy(out=oh8[:, 0:1], in_=pm8[:, 0:1])
                nc.vector.tensor_sub(out=oh8[:, 1:NB2], in0=pm8[:, 1:NB2],
                                     in1=pm8[:, 0:NB2 - 1])
                # ohcol[r] = oh8[n_r, b_r] via PE row-broadcast + masked accum
                psO = pp.tile([pk, NB2], f32, name=f"psO{k}")
                nc.tensor.matmul(psO, mask4t, oh8, start=True, stop=True)
                ohcol = fp.tile([pk, 1], f32, name=f"ohcol{k}")
                junkO = fp.tile([pk, NB2], f32, name=f"junkO{k}")
                nc.vector.scalar_tensor_tensor(
                    out=junkO, in0=psO, scalar=1.0, in1=mb32,
                    op0=AT.mult, op1=AT.mult, accum_out=ohcol)
                ohbd = fp.tile([pk, spt], f32, name=f"ohbd{k}")
                nc.vector.tensor_scalar(out=ohbd, in0=mask4, scalar1=ohcol,
                                        scalar2=None, op0=AT.mult)
                pj = pp.tile([spt, 3 * T], f32, name=f"pj{k}")
                nc.tensor.matmul(pj, ohbd, projs[k], start=True, stop=True)
                outt = fp.tile([spt, T, 3], f32, name=f"outt{k}")
                nc.scalar.copy(out=_view(outt, [[1, 3], [3, T]]),
                               in_=_view(pj, [[T, 3], [1, T]]))
                nc.sync.dma_start(out=out_d.ap()[k * spt:(k + 1) * spt],
                                  in_=outt)
            for k in range(ntiles):
                p = min(RT, rows - k * RT)
                r0 = k * RT

                rpt = wp.tile([p, 3, NP], f32, tag="rpt")
                nc.sync.dma_start(out=rpt, in_=rp_d.ap()[r0:r0 + p])
                mt = wp.tile([p, NP], f32, tag="mt")
                nc.sync.dma_start(out=mt, in_=mk_d.ap()[r0:r0 + p])
                # trajectory of each row's sample, broadcast to its 32 rows
                tpb = wp.tile([p, 3, T], f32, tag="tpb")
                nc.sync.dma_start(out=tpb, in_=bass.AP(
                    tensor=tj_d.ap().tensor, offset=k * spt * 3 * T,
                    ap=[[3 * T, spt], [0, NB2], [1, 3 * T]]))

                # --- segment data ---
                sv = wp.tile([p, 3, NSEG], f32, tag="sv")
                nc.vector.tensor_sub(out=sv, in0=rpt[:, :, 1:NP],
                                     in1=rpt[:, :, 0:NSEG])
                sm = wp.tile([p, NSEG], f32, tag="sm")
                nc.vector.tensor_mul(out=sm, in0=mt[:, 1:NP], in1=mt[:, 0:NSEG])
                sq3 = wp.tile([p, 3, NSEG], f32, tag="sq3")
                nc.vector.tensor_mul(out=sq3, in0=sv, in1=sv)
                sl2 = wp.tile([p, NSEG], f32, tag="sl2")
                nc.vector.tensor_reduce(out=sl2,
                                        in_=_view(sq3, [[1, NSEG], [NSEG, 3]]),
                                        axis=AX.X, op=AT.add)
                sl2m = wp.tile([p, NSEG], f32, tag="sl2m")
                nc.vector.tensor_mul(out=sl2m, in0=sl2, in1=sm)
                sl = wp.tile([p, NSEG], f32, tag="sl")
                eps2 = wp.tile([p, 1], f32, tag="eps2")
                nc.vector.memset(eps2, 1e-18)
                nc.scalar.activation(out=sl, in_=sl2m,
                                     func=mybir.ActivationFunctionType.Sqrt,
                                     bias=eps2)

                zc = wp.tile([p, 1], f32, tag="zc")
                nc.vector.memset(zc, 0.0)
                rscr = wp.tile([p, NSEG], f32, tag="rscr")
                # --- project p0 on all segments; entry_s via one-hot argmin ---
                tmp3 = wp.tile([p, 3, NSEG], f32, tag="tmp3")
                for c in range(3):
                    # (a_c - p0_c) * sv_c
                    nc.vector.scalar_tensor_tensor(
                        out=tmp3[:, c, :], in0=rpt[:, c, 0:NSEG],
                        scalar=tpb[:, c, 0:1], in1=sv[:, c, :],
                        op0=AT.subtract, op1=AT.mult)
                dotn = wp.tile([p, NSEG], f32, tag="dotn")
                nc.vector.tensor_reduce(out=dotn,
                                        in_=_view(tmp3, [[1, NSEG], [NSEG, 3]]),
                                        axis=AX.X, op=AT.add)
                svd = wp.tile([p, NSEG], f32, tag="svd")
                nc.vector.tensor_scalar(out=svd, in0=sl2, scalar1=1e-12,
                                        scalar2=None, op0=AT.max)
                rsvd = wp.tile([p, NSEG], f32, tag="rsvd")
                nc.vector.reciprocal_approx_accurate(out=rsvd, in_=svd,
                                                     scratch=rscr)
                t0 = wp.tile([p, NSEG], f32, tag="t0")
                nc.vector.tensor_mul(out=t0, in0=dotn, in1=rsvd)
                # t0 = min(max(-t0, 0), 1)
                nc.vector.tensor_scalar(out=t0, in0=t0, scalar1=-1.0,
                                        scalar2=0.0, op0=AT.mult, op1=AT.max)
                nc.vector.tensor_scalar(out=t0, in0=t0, scalar1=1.0,
                                        scalar2=None, op0=AT.min)
                s3 = wp.tile([p, 3, NSEG], f32, tag="s3")
                nc.vector.tensor_mul(out=s3, in0=sv,
                                     in1=_view(t0, [[0, 3], [1, NSEG]]))
                e3 = wp.tile([p, 3, NSEG], f32, tag="e3")
                for c in range(3):
                    # (a_c - p0_c) + t0*sv_c  (= q0_c - p0_c)
                    nc.vector.scalar_tensor_tensor(
                        out=e3[:, c, :], in0=rpt[:, c, 0:NSEG],
                        scalar=tpb[:, c, 0:1], in1=s3[:, c, :],
                        op0=AT.subtract, op1=AT.add)
                e3sq = wp.tile([p, 3, NSEG], f32, tag="e3sq")
                nc.vector.tensor_mul(out=e3sq, in0=e3, in1=e3)
                d2 = wp.tile([p, NSEG], f32, tag="d2")
                nc.vector.tensor_reduce(out=d2,
                                        in_=_view(e3sq, [[1, NSEG], [NSEG, 3]]),
                                        axis=AX.X, op=AT.add)
                d2m = wp.tile([p, NSEG], f32, tag="d2m")
                # d2m = d2 + (1-sm)*BIG  (sm is exactly 0/1)
                nc.vector.tensor_scalar(out=d2m, in0=sm, scalar1=1.0,
                                        scalar2=-BIG, op0=AT.subtract,
                                        op1=AT.mult)
                nc.vector.tensor_add(out=d2m, in0=d2m, in1=d2)
                dmin = wp.tile([p, 1], f32, tag="dmin")
                nc.vector.tensor_reduce(out=dmin, in_=d2m, axis=AX.X, op=AT.min)
                ohseg = wp.tile([p, NSEG], f32, tag="ohseg")
                nc.vector.tensor_scalar(out=ohseg, in0=d2m, scalar1=dmin,
                                        scalar2=None, op0=AT.is_equal)
                # keep only the FIRST hot via prefix-max diff (jnp.argmin ties)
                pmax = wp.tile([p, NSEG], f32, tag="pmax")
                nc.vector.tensor_tensor_scan(
                    out=pmax, data0=ohseg, data1=_view(zc, [[0, NSEG]]),
                    initial=0.0, op0=AT.max, op1=AT.add)
                nc.vector.tensor_copy(out=ohseg[:, 0:1], in_=pmax[:, 0:1])
                nc.vector.tensor_sub(out=ohseg[:, 1:NSEG], in0=pmax[:, 1:NSEG],
                                     in1=pmax[:, 0:NSEG - 1])
                cum = wp.tile([p, NP], f32, tag="cum")
                nc.vector.memset(cum[:, 0:1], 0.0)
                nc.vector.tensor_tensor_scan(
                    out=cum[:, 1:NP], data0=sl, data1=_view(zc, [[0, NSEG]]),
                    initial=0.0, op0=AT.add, op1=AT.add)
                total = cum[:, NP - 1:NP]
                # negated cumsum for the hinge stream (on ACT, frees DVE)
                cumneg = wp.tile([p, NSEG], f32, tag="cumneg")
                nc.scalar.activation(out=cumneg, in_=cum[:, 0:NSEG],
                                     func=mybir.ActivationFunctionType.Copy,
                                     scale=-1.0)
                rsl = wp.tile([p, NSEG], f32, tag="rsl")
                nc.vector.reciprocal_approx_accurate(out=rsl, in_=sl,
                                                     scratch=rscr)
                rslm = wp.tile([p, NSEG], f32, tag="rslm")
                nc.vector.tensor_mul(out=rslm, in0=rsl, in1=sm)
                wm = wp.tile([p, 3, NSEG], f32, tag="wm")
                nc.vector.tensor_mul(out=wm, in0=sv,
                                     in1=_view(rslm, [[0, 3], [1, NSEG]]))
                # dw[c,0] = wm[c,0]; dw[c,j] = wm[c,j]-wm[c,j-1]
                dw = wp.tile([p, 3, NSEG], f32, tag="dw")
                nc.scalar.copy(out=_view(dw, [[NSEG, 3]]),
                               in_=_view(wm, [[NSEG, 3]]))
                nc.vector.tensor_sub(
                    out=_view(dw, [[NSEG, 3], [1, NSEG - 1]], extra_off=1),
                    in0=_view(wm, [[NSEG, 3], [1, NSEG - 1]], extra_off=1),
                    in1=_view(wm, [[NSEG, 3], [1, NSEG - 1]]))

                es = wp.tile([p, NSEG], f32, tag="es")
                nc.vector.tensor_mul(out=es, in0=t0, in1=sl)
                nc.vector.tensor_add(out=es, in0=es, in1=cum[:, 0:NSEG])
                entry = wp.tile([p, 1], f32, tag="entry")
                junk0 = wp.tile([p, NSEG], f32, tag="junk0")
                nc.vector.scalar_tensor_tensor(
                    out=junk0, in0=ohseg, scalar=1.0, in1=es,
                    op0=AT.mult, op1=AT.mult, accum_out=entry)

                # --- base point rp[first valid segment] ---
                ohf = wp.tile([p, NSEG], f32, tag="ohf")
                nc.vector.tensor_copy(out=ohf[:, 0:1], in_=sm[:, 0:1])
                nc.vector.tensor_sub(out=ohf[:, 1:NSEG], in0=sm[:, 1:NSEG],
                                     in1=sm[:, 0:NSEG - 1])
                nc.vector.tensor_scalar(out=ohf, in0=ohf, scalar1=0.0,
                                        scalar2=None, op0=AT.max)
                base3 = wp.tile([p, 3], f32, tag="base3")
                for c in range(3):
                    nc.vector.scalar_tensor_tensor(
                        out=junk0, in0=ohf, scalar=1.0, in1=rpt[:, c, 0:NSEG],
                        op0=AT.mult, op1=AT.mult,
                        accum_out=base3[:, c:c + 1])

                # --- trajectory cumulative arc length + target_s ---
                td = wp.tile([p, 3, T - 1], f32, tag="td")
                nc.vector.tensor_sub(out=td, in0=tpb[:, :, 1:T],
                                     in1=tpb[:, :, 0:T - 1])
                td2 = wp.tile([p, 3, T - 1], f32, tag="td2")
                nc.vector.tensor_mul(out=td2, in0=td, in1=td)
                tl2 = wp.tile([p, T - 1], f32, tag="tl2")
                nc.vector.tensor_reduce(out=tl2,
                                        in_=_view(td2, [[1, T - 1], [T - 1, 3]]),
                                        axis=AX.X, op=AT.add)
                tl = wp.tile([p, T - 1], f32, tag="tl")
                nc.scalar.sqrt(out=tl, in_=tl2)
                tcum = wp.tile([p, T], f32, tag="tcum")
                nc.vector.memset(tcum[:, 0:1], 0.0)
                nc.vector.tensor_tensor_scan(
                    out=tcum[:, 1:T], data0=tl, data1=_view(zc, [[0, T - 1]]),
                    initial=0.0, op0=AT.add, op1=AT.add)
                target = wp.tile([p, T], f32, tag="target")
                nc.vector.scalar_tensor_tensor(
                    out=target, in0=tcum, scalar=entry,
                    in1=_view(total, [[0, T]]), op0=AT.add, op1=AT.min)
                nc.vector.tensor_scalar(out=target, in0=target, scalar1=0.0,
                                        scalar2=None, op0=AT.max)

                # --- main pass: fused hinge-scan, one DVE instr per t;
                # the last GP_T timesteps run on Pool (gpsimd) + ACT accum ---
                TD = T - GP_T
                E = wp.tile([p, 3, T], f32, tag="E")
                scrs = [wp.tile([p, 3 * NSEG], f32, name=f"scr{k}_{i}")
                        for i in range(3)]
                cn_b = _view(cumneg, [[0, 3], [1, NSEG]])
                emit_endgame(k - 1)
                if GP_T > 0:
                    Eg = wp.tile([p, 3, GP_T], f32, tag="Eg")
                    vts = [wp.tile([p, NSEG], f32, name=f"vt{k}_{i}")
                           for i in range(2)]
                    mcs = [wp.tile([p, NSEG], f32, name=f"mc{k}_{i}")
                           for i in range(6)]
                for t in range(T):
                    if t < TD:
                        scr = scrs[t % 3]
                        nc.vector._custom_dve(
                            HINGE, out=scr, in0=cn_b, in1=dw,
                            s0=target[:, t:t + 1], s1=0.0)
                        nc.scalar.copy(
                            out=_view(E, [[T, 3]], extra_off=t),
                            in_=_view(scr, [[NSEG, 3]], extra_off=NSEG - 1))
                    else:
                        i = t - TD
                        vt = vts[i % 2]
                        nc.gpsimd.tensor_scalar(
                            out=vt, in0=cumneg, scalar1=target[:, t:t + 1],
                            scalar2=0.0, op0=AT.add, op1=AT.max)
                        for c in range(3):
                            mc = mcs[(3 * i + c) % 6]
                            nc.gpsimd.tensor_mul(out=mc, in0=vt,
                                                 in1=dw[:, c, :])
                            nc.scalar.activation(
                                out=mc, in_=mc,
                                func=mybir.ActivationFunctionType.Copy,
                                accum_out=Eg[:, c, i:i + 1])

                # page-end diffs + base -> proj
                proj = wp.tile([p, 3, T], f32, name=f"proj{k}", bufs=1)
                nc.vector.tensor_scalar(out=proj[:, 0, 0:TD], in0=E[:, 0, 0:TD],
                                        scalar1=base3[:, 0:1], scalar2=None,
                                        op0=AT.add)
                for c in (1, 2):
                    nc.vector.scalar_tensor_tensor(
                        out=proj[:, c, 0:TD], in0=E[:, c, 0:TD],
                        scalar=base3[:, c:c + 1], in1=E[:, c - 1, 0:TD],
                        op0=AT.add, op1=AT.subtract)
                if GP_T > 0:
                    for c in range(3):
                        nc.vector.tensor_scalar(
                            out=proj[:, c, TD:T], in0=Eg[:, c, :],
                            scalar1=base3[:, c:c + 1], scalar2=None,
                            op0=AT.add)
                projs.append(proj)

                # --- cost ---
                df2 = wp.tile([p, 3, T], f32, tag="df2")
                nc.vector._custom_dve(DIFFSQ, out=df2, in0=proj, in1=tpb,
                                      s0=0.0, s1=0.0)
                dd = wp.tile([p, T], f32, tag="dd")
                nc.vector.tensor_reduce(out=dd,
                                        in_=_view(df2, [[1, T], [T, 3]]),
                                        axis=AX.X, op=AT.add)
                dist = wp.tile([p, T], f32, tag="dist")
                cost = wp.tile([p, 1], f32, name=f"cost{k}", bufs=1)
                nc.scalar.activation(out=dist, in_=dd,
                                     func=mybir.ActivationFunctionType.Sqrt,
                                     accum_out=cost)
                costs.append(cost)

            for k in range(ntiles):
                emit_endgame(k)

    nc.compile()
    return nc


def marshal_inputs(selected_traj, road_points, road_mask):
    """Host-side layout marshaling (permutations/casts only): per-core input
    dicts with fwd+bwd branch rows and planar (xyz-major) layouts."""
    st = np.ascontiguousarray(selected_traj, dtype=np.float32)
    rp = np.ascontiguousarray(road_points, dtype=np.float32)
    rm = np.asarray(road_mask)

    rp_ext = np.concatenate([rp, rp[:, :, ::-1, :]], axis=1)        # [N,NB2,NP,3]
    rp_ext = np.ascontiguousarray(rp_ext.transpose(0, 1, 3, 2))     # [N,NB2,3,NP]
    mk_ext = np.concatenate([rm, rm[:, :, ::-1]], axis=1).astype(np.float32)
    tj = np.ascontiguousarray(st.transpose(0, 2, 1))                # [N,3,T]

    spt = RT // NB2
    m4 = np.zeros((RT, spt), dtype=np.float32)
    for s in range(spt):
        m4[s * NB2:(s + 1) * NB2, s] = 1.0
    m4t = np.ascontiguousarray(m4.T)
    mb = np.zeros((RT, NB2), dtype=np.float32)
    mb[np.arange(RT), np.arange(RT) % NB2] = 1.0

    in_maps = []
    for c in range(NCORES):
        s = slice(c * NS, (c + 1) * NS)
        in_maps.append({
            "rp": np.ascontiguousarray(rp_ext[s]).reshape(NS * NB2, 3, NP),
            "mk": np.ascontiguousarray(mk_ext[s]).reshape(NS * NB2, NP),
            "tj": np.ascontiguousarray(tj[s]),
            "m4": m4,
            "m4t": m4t,
            "mb": mb,
        })
    return in_maps


_NC = None


def kernel(selected_traj, road_points, road_mask):
    global _NC
    if _NC is None:
        _NC = build_nc()
    in_maps = marshal_inputs(selected_traj, road_points, road_mask)
    res = run_bass_kernel_spmd(_NC, in_maps, core_ids=list(range(NCORES)))
    out = np.concatenate([r["out"] for r in res.results], axis=0)
    return out.astype(np.float32)


# revision 18
# speedup vs baseline: 1.0634x; 1.0049x over previous
"""Trainium2 Bass kernel for the arc-projection problem.

Full-input contract: kernel(**inputs) takes the unsharded numpy inputs and
returns the full output. Internally shards the batch N=64 across 8 cores
(pure data parallel), runs one SPMD Bass kernel, and gathers.

Algorithm (matches reference._arc_projection, reformulated gather-free):
  For each (sample, branch, direction) row:
    - segment vectors sv, masked lengths sl, cumsum cum, masked unit
      tangents wm = sv/sl*mask, and their first differences dw.
    - project trajectory point 0 on all segments -> entry_s (one-hot argmin)
    - target_s[t] = clip(entry_s + traj_cum[t], 0, total)
    - proj_c(s) = base_c + sum_j dw_c[j] * relu(s - cum_j)   (telescoped
      hinge; equals the reference's searchsorted+lerp for prefix/suffix
      masks). Computed by a custom fused DVE op: one instruction per t
      producing running sums whose page-ends give all 3 channels.
    - cost = sum_t |pos_t - proj_t|; per-sample argmin over 32 rows via
      one-hot; best-branch gather via a PE matmul with a block-diagonal
      one-hot stationary (no DRAM round-trip).
"""

import sys

import numpy as np

try:
    import concourse.bass as bass
except ImportError:  # pragma: no cover - container without PYTHONPATH set
    sys.path.insert(0, "/opt/trn_rl_repo")
    import concourse.bass as bass

import concourse.tile as tile
from concourse import bacc, mybir
from concourse.bass_utils import run_bass_kernel_spmd

import concourse.dve_ops as _dmod
from concourse.dve_spec import (
    Spec, Src0, Src1, C0, relu, scan,
    AluOp as _AluOp, lower as _dve_lower, _has_src1,
)
from concourse.dve_uop import DveOpSpec as _DveOpSpec

f32 = mybir.dt.float32
AT = mybir.AluOpType
AX = mybir.AxisListType

N, T, NB, NP = 64, 128, 16, 256
NCORES = 8
NS = N // NCORES          # samples per core
NB2 = 2 * NB              # fwd + bwd branches
NSEG = NP - 1
NS2 = NP                  # padded segment count (256) for the hinge stream
BIG = 1.0e30
RT = 128                  # rows per partition-tile
GP_T = 0                  # Pool offload disabled: SBUF-port contention with DVE


def _register_hinge_op():
    """out[p,k] = cumsum_k(relu(in0[p,k] + s0[p]) * in1[p,k]) as one DVE op."""
    name = "HINGE_SCAN_ANT"
    for o in _dmod.OPS:
        if o.name == name:
            return o

    def _ref(in0, in1, s0, s1, imm2):
        x = (np.maximum(in0.astype(np.float32) + s0, 0.0) * in1).astype(np.float32)
        xf = x.reshape(x.shape[0], -1)
        return np.cumsum(xf, axis=-1).astype(np.float32).reshape(x.shape)

    spec = Spec(body=scan(_AluOp.ADD, relu(Src0 + C0) * Src1), reference=_ref)
    op = _dmod.DveOp(name, spec, subdim=False, uops_sha={})
    _dmod.OPS.append(op)
    _dmod._SUB_OPCODE_FOR_NAME[name] = _dmod._CUSTOM_DVE_ROW_BASE + len(_dmod.OPS) - 1
    for ver in ("v3", "v4"):
        _dmod._COMPILE_CACHE[(name, ver)] = _DveOpSpec(
            name=name,
            opcode=_dmod._SUB_OPCODE_FOR_NAME[name],
            uops=_dve_lower(spec, ver=ver),
            rd1_en=_has_src1(spec),
        )
    return op


HINGE = _register_hinge_op()


def _register_diffsq_op():
    """out[p,k] = (in0[p,k] - in1[p,k])**2 as one DVE op."""
    name = "DIFFSQ_ANT"
    for o in _dmod.OPS:
        if o.name == name:
            return o
    from concourse.dve_spec import sq as _sq
    spec = Spec(
        body=_sq(Src0 - Src1),
        reference=lambda in0, in1, s0, s1, imm2:
            ((in0.astype(np.float32) - in1) ** 2).astype(np.float32),
    )
    op = _dmod.DveOp(name, spec, subdim=False, uops_sha={})
    _dmod.OPS.append(op)
    _dmod._SUB_OPCODE_FOR_NAME[name] = _dmod._CUSTOM_DVE_ROW_BASE + len(_dmod.OPS) - 1
    for ver in ("v3", "v4"):
        _dmod._COMPILE_CACHE[(name, ver)] = _DveOpSpec(
            name=name,
            opcode=_dmod._SUB_OPCODE_FOR_NAME[name],
            uops=_dve_lower(spec, ver=ver),
            rd1_en=_has_src1(spec),
        )
    return op


DIFFSQ = _register_diffsq_op()


def _view(t, ap_dims, extra_off=0):
    """Strided view of a tile/AP: ap_dims are [step, count] free dims after
    the partition dim (kept from t)."""
    return bass.AP(tensor=t.tensor, offset=t.offset + extra_off,
                   ap=[t.ap[0]] + ap_dims)


def _dview(t, ap_dims, extra_off=0):
    """Raw view of a DRAM tile: ap_dims replace all dims."""
    return bass.AP(tensor=t.tensor, offset=t.offset + extra_off, ap=ap_dims)


def build_nc(ns=NS, enable_asserts=False):
    rows = ns * NB2
    ntiles = (rows + RT - 1) // RT
    spt = RT // NB2  # samples per tile

    nc = bacc.Bacc("TRN2", target_bir_lowering=False, debug=False,
                   enable_asserts=enable_asserts, num_devices=NCORES)

    rp_d = nc.dram_tensor("rp", [rows, 3, NP], f32, kind="ExternalInput")
    mk_d = nc.dram_tensor("mk", [rows, NP], f32, kind="ExternalInput")
    tj_d = nc.dram_tensor("tj", [ns, 3, T], f32, kind="ExternalInput")
    m4_d = nc.dram_tensor("m4", [RT, spt], f32, kind="ExternalInput")
    m4t_d = nc.dram_tensor("m4t", [RT // NB2, RT], f32, kind="ExternalInput")
    mb_d = nc.dram_tensor("mb", [RT, NB2], f32, kind="ExternalInput")
    out_d = nc.dram_tensor("out", [ns, T, 3], f32, kind="ExternalOutput")

    with tile.TileContext(nc) as tc:
        with (
            tc.tile_pool(name="work", bufs=2) as wp,
            tc.tile_pool(name="fin", bufs=1) as fp,
            tc.tile_pool(name="ps", bufs=1, space="PSUM") as pp,
            tc.tile_pool(name="dram", bufs=1, space="DRAM") as dp,
        ):
            mask4 = fp.tile([RT, spt], f32)
            nc.gpsimd.dma_start(out=mask4, in_=m4_d.ap())
            mask4t = fp.tile([spt, RT], f32)
            nc.gpsimd.dma_start(out=mask4t, in_=m4t_d.ap())
            mb32 = fp.tile([RT, NB2], f32)
            nc.gpsimd.dma_start(out=mb32, in_=mb_d.ap())

            projs = []
            costs = []
            done_endgame = set()

            def emit_endgame(k):
                if k < 0 or k >= len(projs) or k in done_endgame:
                    return
                done_endgame.add(k)
                pk = min(RT, rows - k * RT)
                rk = k * RT
                # costT = mask4^T @ (cost * mb32): [4, 32] sample-major, on-chip
                costm = fp.tile([pk, NB2], f32, name=f"costm{k}")
                nc.vector.tensor_scalar(out=costm, in0=mb32,
                                        scalar1=costs[k], scalar2=None,
                                        op0=AT.mult)
                psT = pp.tile([spt, NB2], f32, name=f"psT{k}")
                nc.tensor.matmul(psT, mask4, costm, start=True, stop=True)
                costT = fp.tile([spt, NB2], f32, name=f"costT{k}")
                nc.scalar.copy(out=costT, in_=psT)
                cmin = fp.tile([spt, 1], f32, name=f"cmin{k}")
                nc.vector.tensor_reduce(out=cmin, in_=costT, axis=AX.X,
                                        op=AT.min)
                oh8 = fp.tile([spt, NB2], f32, name=f"oh8{k}")
                nc.vector.tensor_scalar(out=oh8, in0=costT, scalar1=cmin,
                                        scalar2=None, op0=AT.is_equal)
                zc8 = fp.tile([spt, 1], f32, name=f"zc8{k}")
                nc.vector.memset(zc8, 0.0)
                pm8 = fp.tile([spt, NB2], f32, name=f"pm8{k}")
                nc.vector.tensor_tensor_scan(
                    out=pm8, data0=oh8, data1=_view(zc8, [[0, NB2]]),
                    initial=0.0, op0=AT.max, op1=AT.add)
                nc.vector.tensor_copy(out=oh8[:, 0:1], in_=pm8[:, 0:1])
                nc.vector.tensor_sub(out=oh8[:, 1:NB2], in0=pm8[:, 1:NB2],
                                     in1=pm8[:, 0:NB2 - 1])
                # ohcol[r] = oh8[n_r, b_r] via PE row-broadcast + masked accum
                psO = pp.tile([pk, NB2], f32, name=f"psO{k}")
                nc.tensor.matmul(psO, mask4t, oh8, start=True, stop=True)
                ohcol = fp.tile([pk, 1], f32, name=f"ohcol{k}")
                junkO = fp.tile([pk, NB2], f32, name=f"junkO{k}")
                nc.vector.scalar_tensor_tensor(
                    out=junkO, in0=psO, scalar=1.0, in1=mb32,
                    op0=AT.mult, op1=AT.mult, accum_out=ohcol)
                ohbd = fp.tile([pk, spt], f32, name=f"ohbd{k}")
                nc.vector.tensor_scalar(out=ohbd, in0=mask4, scalar1=ohcol,
                                        scalar2=None, op0=AT.mult)
                pj = pp.tile([spt, 3 * T], f32, name=f"pj{k}")
                nc.tensor.matmul(pj, ohbd, projs[k], start=True, stop=True)
                outt = fp.tile([spt, T, 3], f32, name=f"outt{k}")
                nc.scalar.copy(out=_view(outt, [[1, 3], [3, T]]),
                               in_=_view(pj, [[T, 3], [1, T]]))
                nc.sync.dma_start(out=out_d.ap()[k * spt:(k + 1) * spt],
                                  in_=outt)
            for k in range(ntiles):
                p = min(RT, rows - k * RT)
                r0 = k * RT

                rpt = wp.tile([p, 3, NP], f32, tag="rpt")
                nc.sync.dma_start(out=rpt, in_=rp_d.ap()[r0:r0 + p])
                mt = wp.tile([p, NP], f32, tag="mt")
                nc.gpsimd.dma_start(out=mt, in_=mk_d.ap()[r0:r0 + p])
                # trajectory of each row's sample, broadcast to its 32 rows
                tpb = wp.tile([p, 3, T], f32, tag="tpb")
                nc.scalar.dma_start(out=tpb, in_=bass.AP(
                    tensor=tj_d.ap().tensor, offset=k * spt * 3 * T,
                    ap=[[3 * T, spt], [0, NB2], [1, 3 * T]]))

                # --- segment data ---
                sv = wp.tile([p, 3, NSEG], f32, tag="sv")
                nc.vector.tensor_sub(out=sv, in0=rpt[:, :, 1:NP],
                                     in1=rpt[:, :, 0:NSEG])
                sm = wp.tile([p, NSEG], f32, tag="sm")
                nc.vector.tensor_mul(out=sm, in0=mt[:, 1:NP], in1=mt[:, 0:NSEG])
                sq3 = wp.tile([p, 3, NSEG], f32, tag="sq3")
                nc.vector.tensor_mul(out=sq3, in0=sv, in1=sv)
                sl2 = wp.tile([p, NSEG], f32, tag="sl2")
                nc.vector.tensor_reduce(out=sl2,
                                        in_=_view(sq3, [[1, NSEG], [NSEG, 3]]),
                                        axis=AX.X, op=AT.add)
                sl2m = wp.tile([p, NSEG], f32, tag="sl2m")
                nc.vector.tensor_mul(out=sl2m, in0=sl2, in1=sm)
                sl = wp.tile([p, NSEG], f32, tag="sl")
                eps2 = wp.tile([p, 1], f32, tag="eps2")
                nc.vector.memset(eps2, 1e-18)
                nc.scalar.activation(out=sl, in_=sl2m,
                                     func=mybir.ActivationFunctionType.Sqrt,
                                     bias=eps2)

                zc = wp.tile([p, 1], f32, tag="zc")
                nc.vector.memset(zc, 0.0)
                rscr = wp.tile([p, NSEG], f32, tag="rscr")
                # --- project p0 on all segments; entry_s via one-hot argmin ---
                tmp3 = wp.tile([p, 3, NSEG], f32, tag="tmp3")
                for c in range(3):
                    # (a_c - p0_c) * sv_c
                    nc.vector.scalar_tensor_tensor(
                        out=tmp3[:, c, :], in0=rpt[:, c, 0:NSEG],
                        scalar=tpb[:, c, 0:1], in1=sv[:, c, :],
                        op0=AT.subtract, op1=AT.mult)
                dotn = wp.tile([p, NSEG], f32, tag="dotn")
                nc.vector.tensor_reduce(out=dotn,
                                        in_=_view(tmp3, [[1, NSEG], [NSEG, 3]]),
                                        axis=AX.X, op=AT.add)
                svd = wp.tile([p, NSEG], f32, tag="svd")
                nc.vector.tensor_scalar(out=svd, in0=sl2, scalar1=1e-12,
                                        scalar2=None, op0=AT.max)
                rsvd = wp.tile([p, NSEG], f32, tag="rsvd")
                nc.vector.reciprocal_approx_accurate(out=rsvd, in_=svd,
                                                     scratch=rscr)
                t0 = wp.tile([p, NSEG], f32, tag="t0")
                nc.vector.tensor_mul(out=t0, in0=dotn, in1=rsvd)
                # t0 = min(max(-t0, 0), 1)
                nc.vector.tensor_scalar(out=t0, in0=t0, scalar1=-1.0,
                                        scalar2=0.0, op0=AT.mult, op1=AT.max)
                nc.vector.tensor_scalar(out=t0, in0=t0, scalar1=1.0,
                                        scalar2=None, op0=AT.min)
                s3 = wp.tile([p, 3, NSEG], f32, tag="s3")
                nc.vector.tensor_mul(out=s3, in0=sv,
                                     in1=_view(t0, [[0, 3], [1, NSEG]]))
                e3 = wp.tile([p, 3, NSEG], f32, tag="e3")
                for c in range(3):
                    # (a_c - p0_c) + t0*sv_c  (= q0_c - p0_c)
                    nc.vector.scalar_tensor_tensor(
                        out=e3[:, c, :], in0=rpt[:, c, 0:NSEG],
                        scalar=tpb[:, c, 0:1], in1=s3[:, c, :],
                        op0=AT.subtract, op1=AT.add)
                e3sq = wp.tile([p, 3, NSEG], f32, tag="e3sq")
                nc.vector.tensor_mul(out=e3sq, in0=e3, in1=e3)
                d2 = wp.tile([p, NSEG], f32, tag="d2")
                nc.vector.tensor_reduce(out=d2,
                                        in_=_view(e3sq, [[1, NSEG], [NSEG, 3]]),
                                        axis=AX.X, op=AT.add)
                d2m = wp.tile([p, NSEG], f32, tag="d2m")
                # d2m = d2 + (1-sm)*BIG  (sm is exactly 0/1)
                nc.vector.tensor_scalar(out=d2m, in0=sm, scalar1=1.0,
                                        scalar2=-BIG, op0=AT.subtract,
                                        op1=AT.mult)
                nc.vector.tensor_add(out=d2m, in0=d2m, in1=d2)
                dmin = wp.tile([p, 1], f32, tag="dmin")
                nc.vector.tensor_reduce(out=dmin, in_=d2m, axis=AX.X, op=AT.min)
                ohseg = wp.tile([p, NSEG], f32, tag="ohseg")
                nc.vector.tensor_scalar(out=ohseg, in0=d2m, scalar1=dmin,
                                        scalar2=None, op0=AT.is_equal)
                # keep only the FIRST hot via prefix-max diff (jnp.argmin ties)
                pmax = wp.tile([p, NSEG], f32, tag="pmax")
                nc.vector.tensor_tensor_scan(
                    out=pmax, data0=ohseg, data1=_view(zc, [[0, NSEG]]),
                    initial=0.0, op0=AT.max, op1=AT.add)
                nc.vector.tensor_copy(out=ohseg[:, 0:1], in_=pmax[:, 0:1])
                nc.vector.tensor_sub(out=ohseg[:, 1:NSEG], in0=pmax[:, 1:NSEG],
                                     in1=pmax[:, 0:NSEG - 1])
                cum = wp.tile([p, NP], f32, tag="cum")
                nc.vector.memset(cum[:, 0:1], 0.0)
                nc.vector.tensor_tensor_scan(
                    out=cum[:, 1:NP], data0=sl, data1=_view(zc, [[0, NSEG]]),
                    initial=0.0, op0=AT.add, op1=AT.add)
                total = cum[:, NP - 1:NP]
                # negated cumsum for the hinge stream (on ACT, frees DVE)
                cumneg = wp.tile([p, NSEG], f32, tag="cumneg")
                nc.scalar.activation(out=cumneg, in_=cum[:, 0:NSEG],
                                     func=mybir.ActivationFunctionType.Copy,
                                     scale=-1.0)
                rsl = wp.tile([p, NSEG], f32, tag="rsl")
                nc.vector.reciprocal_approx_accurate(out=rsl, in_=sl,
                                                     scratch=rscr)
                rslm = wp.tile([p, NSEG], f32, tag="rslm")
                nc.vector.tensor_mul(out=rslm, in0=rsl, in1=sm)
                wm = wp.tile([p, 3, NSEG], f32, tag="wm")
                nc.vector.tensor_mul(out=wm, in0=sv,
                                     in1=_view(rslm, [[0, 3], [1, NSEG]]))
                # dw[c,0] = wm[c,0]; dw[c,j] = wm[c,j]-wm[c,j-1]
                dw = wp.tile([p, 3, NSEG], f32, tag="dw")
                nc.scalar.copy(out=_view(dw, [[NSEG, 3]]),
                               in_=_view(wm, [[NSEG, 3]]))
                nc.vector.tensor_sub(
                    out=_view(dw, [[NSEG, 3], [1, NSEG - 1]], extra_off=1),
                    in0=_view(wm, [[NSEG, 3], [1, NSEG - 1]], extra_off=1),
                    in1=_view(wm, [[NSEG, 3], [1, NSEG - 1]]))

                es = wp.tile([p, NSEG], f32, tag="es")
                nc.vector.tensor_mul(out=es, in0=t0, in1=sl)
                nc.vector.tensor_add(out=es, in0=es, in1=cum[:, 0:NSEG])
                entry = wp.tile([p, 1], f32, tag="entry")
                junk0 = wp.tile([p, NSEG], f32, tag="junk0")
                nc.vector.scalar_tensor_tensor(
                    out=junk0, in0=ohseg, scalar=1.0, in1=es,
                    op0=AT.mult, op1=AT.mult, accum_out=entry)

                # --- base point rp[first valid segment] ---
                ohf = wp.tile([p, NSEG], f32, tag="ohf")
                nc.vector.tensor_copy(out=ohf[:, 0:1], in_=sm[:, 0:1])
                nc.vector.tensor_sub(out=ohf[:, 1:NSEG], in0=sm[:, 1:NSEG],
                                     in1=sm[:, 0:NSEG - 1])
                nc.vector.tensor_scalar(out=ohf, in0=ohf, scalar1=0.0,
                                        scalar2=None, op0=AT.max)
                base3 = wp.tile([p, 3], f32, tag="base3")
                for c in range(3):
                    nc.vector.scalar_tensor_tensor(
                        out=junk0, in0=ohf, scalar=1.0, in1=rpt[:, c, 0:NSEG],
                        op0=AT.mult, op1=AT.mult,
                        accum_out=base3[:, c:c + 1])

                # --- trajectory cumulative arc length + target_s ---
                td = wp.tile([p, 3, T - 1], f32, tag="td")
                nc.vector.tensor_sub(out=td, in0=tpb[:, :, 1:T],
                                     in1=tpb[:, :, 0:T - 1])
                td2 = wp.tile([p, 3, T - 1], f32, tag="td2")
                nc.vector.tensor_mul(out=td2, in0=td, in1=td)
                tl2 = wp.tile([p, T - 1], f32, tag="tl2")
                nc.vector.tensor_reduce(out=tl2,
                                        in_=_view(td2, [[1, T - 1], [T - 1, 3]]),
                                        axis=AX.X, op=AT.add)
                tl = wp.tile([p, T - 1], f32, tag="tl")
                nc.scalar.sqrt(out=tl, in_=tl2)
                tcum = wp.tile([p, T], f32, tag="tcum")
                nc.vector.memset(tcum[:, 0:1], 0.0)
                nc.vector.tensor_tensor_scan(
                    out=tcum[:, 1:T], data0=tl, data1=_view(zc, [[0, T - 1]]),
                    initial=0.0, op0=AT.add, op1=AT.add)
                target = wp.tile([p, T], f32, tag="target")
                nc.vector.scalar_tensor_tensor(
                    out=target, in0=tcum, scalar=entry,
                    in1=_view(total, [[0, T]]), op0=AT.add, op1=AT.min)
                nc.vector.tensor_scalar(out=target, in0=target, scalar1=0.0,
                                        scalar2=None, op0=AT.max)

                # --- main pass: fused hinge-scan, one DVE instr per t;
                # the last GP_T timesteps run on Pool (gpsimd) + ACT accum ---
                TD = T - GP_T
                E = wp.tile([p, 3, T], f32, tag="E")
                scrs = [wp.tile([p, 3 * NSEG], f32, name=f"scr{k}_{i}")
                        for i in range(3)]
                cn_b = _view(cumneg, [[0, 3], [1, NSEG]])
                emit_endgame(k - 1)
                if GP_T > 0:
                    Eg = wp.tile([p, 3, GP_T], f32, tag="Eg")
                    vts = [wp.tile([p, NSEG], f32, name=f"vt{k}_{i}")
                           for i in range(2)]
                    mcs = [wp.tile([p, NSEG], f32, name=f"mc{k}_{i}")
                           for i in range(6)]
                for t in range(T):
                    if t < TD:
                        scr = scrs[t % 3]
                        nc.vector._custom_dve(
                            HINGE, out=scr, in0=cn_b, in1=dw,
                            s0=target[:, t:t + 1], s1=0.0)
                        nc.scalar.copy(
                            out=_view(E, [[T, 3]], extra_off=t),
                            in_=_view(scr, [[NSEG, 3]], extra_off=NSEG - 1))
                    else:
                        i = t - TD
                        vt = vts[i % 2]
                        nc.gpsimd.tensor_scalar(
                            out=vt, in0=cumneg, scalar1=target[:, t:t + 1],
                            scalar2=0.0, op0=AT.add, op1=AT.max)
                        for c in range(3):
                            mc = mcs[(3 * i + c) % 6]
                            nc.gpsimd.tensor_mul(out=mc, in0=vt,
                                                 in1=dw[:, c, :])
                            nc.scalar.activation(
                                out=mc, in_=mc,
                                func=mybir.ActivationFunctionType.Copy,
                                accum_out=Eg[:, c, i:i + 1])

                # page-end diffs + base -> proj
                proj = wp.tile([p, 3, T], f32, name=f"proj{k}", bufs=1)
                nc.vector.tensor_scalar(out=proj[:, 0, 0:TD], in0=E[:, 0, 0:TD],
                                        scalar1=base3[:, 0:1], scalar2=None,
                                        op0=AT.add)
                for c in (1, 2):
                    nc.vector.scalar_tensor_tensor(
                        out=proj[:, c, 0:TD], in0=E[:, c, 0:TD],
                        scalar=base3[:, c:c + 1], in1=E[:, c - 1, 0:TD],
                        op0=AT.add, op1=AT.subtract)
                if GP_T > 0:
                    for c in range(3):
                        nc.vector.tensor_scalar(
                            out=proj[:, c, TD:T], in0=Eg[:, c, :],
                            scalar1=base3[:, c:c + 1], scalar2=None,
                            op0=AT.add)
                projs.append(proj)

                # --- cost ---
                df2 = wp.tile([p, 3, T], f32, tag="df2")
                nc.vector._custom_dve(DIFFSQ, out=df2, in0=proj, in1=tpb,
                                      s0=0.0, s1=0.0)
                dd = wp.tile([p, T], f32, tag="dd")
                nc.vector.tensor_reduce(out=dd,
                                        in_=_view(df2, [[1, T], [T, 3]]),
                                        axis=AX.X, op=AT.add)
                dist = wp.tile([p, T], f32, tag="dist")
                cost = wp.tile([p, 1], f32, name=f"cost{k}", bufs=1)
                nc.scalar.activation(out=dist, in_=dd,
                                     func=mybir.ActivationFunctionType.Sqrt,
                                     accum_out=cost)
                costs.append(cost)

            for k in range(ntiles):
                emit_endgame(k)

    nc.compile()
    return nc


def marshal_inputs(selected_traj, road_points, road_mask):
    """Host-side layout marshaling (permutations/casts only): per-core input
    dicts with fwd+bwd branch rows and planar (xyz-major) layouts."""
    st = np.ascontiguousarray(selected_traj, dtype=np.float32)
    rp = np.ascontiguousarray(road_points, dtype=np.float32)
    rm = np.asarray(road_mask)

    rp_ext = np.concatenate([rp, rp[:, :, ::-1, :]], axis=1)        # [N,NB2,NP,3]
    rp_ext = np.ascontiguousarray(rp_ext.transpose(0, 1, 3, 2))     # [N,NB2,3,NP]
    mk_ext = np.concatenate([rm, rm[:, :, ::-1]], axis=1).astype(np.float32)
    tj = np.ascontiguousarray(st.transpose(0, 2, 1))                # [N,3,T]

    spt = RT // NB2
    m4 = np.zeros((RT, spt), dtype=np.float32)
    for s in range(spt):
        m4[s * NB2:(s + 1) * NB2, s] = 1.0
    m4t = np.ascontiguousarray(m4.T)
    mb = np.zeros((RT, NB2), dtype=np.float32)
    mb[np.arange(RT), np.arange(RT) % NB2] = 1.0

    in_maps = []
    for c in range(NCORES):
        s = slice(c * NS, (c + 1) * NS)
        in_maps.append({
            "rp": np.ascontiguousarray(rp_ext[s]).reshape(NS * NB2, 3, NP),
            "mk": np.ascontiguousarray(mk_ext[s]).reshape(NS * NB2, NP),
            "tj": np.ascontiguousarray(tj[s]),
            "m4": m4,
            "m4t": m4t,
            "mb": mb,
        })
    return in_maps


_NC = None


def kernel(selected_traj, road_points, road_mask):
    global _NC
    if _NC is None:
        _NC = build_nc()
    in_maps = marshal_inputs(selected_traj, road_points, road_mask)
    res = run_bass_kernel_spmd(_NC, in_maps, core_ids=list(range(NCORES)))
    out = np.concatenate([r["out"] for r in res.results], axis=0)
    return out.astype(np.float32)


# revision 19
# speedup vs baseline: 1.0637x; 1.0003x over previous
"""Trainium2 Bass kernel for the arc-projection problem.

Full-input contract: kernel(**inputs) takes the unsharded numpy inputs and
returns the full output. Internally shards the batch N=64 across 8 cores
(pure data parallel), runs one SPMD Bass kernel, and gathers.

Algorithm (matches reference._arc_projection, reformulated gather-free):
  For each (sample, branch, direction) row:
    - segment vectors sv, masked lengths sl, cumsum cum, masked unit
      tangents wm = sv/sl*mask, and their first differences dw.
    - project trajectory point 0 on all segments -> entry_s (one-hot argmin)
    - target_s[t] = clip(entry_s + traj_cum[t], 0, total)
    - proj_c(s) = base_c + sum_j dw_c[j] * relu(s - cum_j)   (telescoped
      hinge; equals the reference's searchsorted+lerp for prefix/suffix
      masks). Computed by a custom fused DVE op: one instruction per t
      producing running sums whose page-ends give all 3 channels.
    - cost = sum_t |pos_t - proj_t|; per-sample argmin over 32 rows via
      one-hot; best-branch gather via a PE matmul with a block-diagonal
      one-hot stationary (no DRAM round-trip).
"""

import sys

import numpy as np

try:
    import concourse.bass as bass
except ImportError:  # pragma: no cover - container without PYTHONPATH set
    sys.path.insert(0, "/opt/trn_rl_repo")
    import concourse.bass as bass

import concourse.tile as tile
from concourse import bacc, mybir
from concourse.bass_utils import run_bass_kernel_spmd

import concourse.dve_ops as _dmod
from concourse.dve_spec import (
    Spec, Src0, Src1, C0, relu, scan,
    AluOp as _AluOp, lower as _dve_lower, _has_src1,
)
from concourse.dve_uop import DveOpSpec as _DveOpSpec

f32 = mybir.dt.float32
AT = mybir.AluOpType
AX = mybir.AxisListType

N, T, NB, NP = 64, 128, 16, 256
NCORES = 8
NS = N // NCORES          # samples per core
NB2 = 2 * NB              # fwd + bwd branches
NSEG = NP - 1
NS2 = NP                  # padded segment count (256) for the hinge stream
BIG = 1.0e30
RT = 128                  # rows per partition-tile
GP_T = 0                  # Pool offload disabled: SBUF-port contention with DVE


def _register_hinge_op():
    """out[p,k] = cumsum_k(relu(in0[p,k] + s0[p]) * in1[p,k]) as one DVE op."""
    name = "HINGE_SCAN_ANT"
    for o in _dmod.OPS:
        if o.name == name:
            return o

    def _ref(in0, in1, s0, s1, imm2):
        x = (np.maximum(in0.astype(np.float32) + s0, 0.0) * in1).astype(np.float32)
        xf = x.reshape(x.shape[0], -1)
        return np.cumsum(xf, axis=-1).astype(np.float32).reshape(x.shape)

    spec = Spec(body=scan(_AluOp.ADD, relu(Src0 + C0) * Src1), reference=_ref)
    op = _dmod.DveOp(name, spec, subdim=False, uops_sha={})
    _dmod.OPS.append(op)
    _dmod._SUB_OPCODE_FOR_NAME[name] = _dmod._CUSTOM_DVE_ROW_BASE + len(_dmod.OPS) - 1
    for ver in ("v3", "v4"):
        _dmod._COMPILE_CACHE[(name, ver)] = _DveOpSpec(
            name=name,
            opcode=_dmod._SUB_OPCODE_FOR_NAME[name],
            uops=_dve_lower(spec, ver=ver),
            rd1_en=_has_src1(spec),
        )
    return op


HINGE = _register_hinge_op()


def _register_diffsq_op():
    """out[p,k] = (in0[p,k] - in1[p,k])**2 as one DVE op."""
    name = "DIFFSQ_ANT"
    for o in _dmod.OPS:
        if o.name == name:
            return o
    from concourse.dve_spec import sq as _sq
    spec = Spec(
        body=_sq(Src0 - Src1),
        reference=lambda in0, in1, s0, s1, imm2:
            ((in0.astype(np.float32) - in1) ** 2).astype(np.float32),
    )
    op = _dmod.DveOp(name, spec, subdim=False, uops_sha={})
    _dmod.OPS.append(op)
    _dmod._SUB_OPCODE_FOR_NAME[name] = _dmod._CUSTOM_DVE_ROW_BASE + len(_dmod.OPS) - 1
    for ver in ("v3", "v4"):
        _dmod._COMPILE_CACHE[(name, ver)] = _DveOpSpec(
            name=name,
            opcode=_dmod._SUB_OPCODE_FOR_NAME[name],
            uops=_dve_lower(spec, ver=ver),
            rd1_en=_has_src1(spec),
        )
    return op


DIFFSQ = _register_diffsq_op()


def _view(t, ap_dims, extra_off=0):
    """Strided view of a tile/AP: ap_dims are [step, count] free dims after
    the partition dim (kept from t)."""
    return bass.AP(tensor=t.tensor, offset=t.offset + extra_off,
                   ap=[t.ap[0]] + ap_dims)


def _dview(t, ap_dims, extra_off=0):
    """Raw view of a DRAM tile: ap_dims replace all dims."""
    return bass.AP(tensor=t.tensor, offset=t.offset + extra_off, ap=ap_dims)


def build_nc(ns=NS, enable_asserts=False):
    rows = ns * NB2
    ntiles = (rows + RT - 1) // RT
    spt = RT // NB2  # samples per tile

    nc = bacc.Bacc("TRN2", target_bir_lowering=False, debug=False,
                   enable_asserts=enable_asserts, num_devices=NCORES)

    rp_d = nc.dram_tensor("rp", [rows, 3, NP], f32, kind="ExternalInput")
    mk_d = nc.dram_tensor("mk", [rows, NP], f32, kind="ExternalInput")
    tj_d = nc.dram_tensor("tj", [ns, 3, T], f32, kind="ExternalInput")
    m4_d = nc.dram_tensor("m4", [RT, spt], f32, kind="ExternalInput")
    m4t_d = nc.dram_tensor("m4t", [RT // NB2, RT], f32, kind="ExternalInput")
    mb_d = nc.dram_tensor("mb", [RT, NB2], f32, kind="ExternalInput")
    out_d = nc.dram_tensor("out", [ns, T, 3], f32, kind="ExternalOutput")

    with tile.TileContext(nc) as tc:
        with (
            tc.tile_pool(name="work", bufs=2) as wp,
            tc.tile_pool(name="fin", bufs=1) as fp,
            tc.tile_pool(name="ps", bufs=1, space="PSUM") as pp,
            tc.tile_pool(name="dram", bufs=1, space="DRAM") as dp,
        ):
            mask4 = fp.tile([RT, spt], f32)
            nc.gpsimd.dma_start(out=mask4, in_=m4_d.ap())
            mask4t = fp.tile([spt, RT], f32)
            nc.gpsimd.dma_start(out=mask4t, in_=m4t_d.ap())
            mb32 = fp.tile([RT, NB2], f32)
            nc.gpsimd.dma_start(out=mb32, in_=mb_d.ap())

            projs = []
            costs = []
            done_endgame = set()

            def emit_endgame(k):
                if k < 0 or k >= len(projs) or k in done_endgame:
                    return
                done_endgame.add(k)
                pk = min(RT, rows - k * RT)
                rk = k * RT
                # costT = mask4^T @ (cost * mb32): [4, 32] sample-major, on-chip
                costm = fp.tile([pk, NB2], f32, name=f"costm{k}")
                nc.vector.tensor_scalar(out=costm, in0=mb32,
                                        scalar1=costs[k], scalar2=None,
                                        op0=AT.mult)
                psT = pp.tile([spt, NB2], f32, name=f"psT{k}")
                nc.tensor.matmul(psT, mask4, costm, start=True, stop=True)
                costT = fp.tile([spt, NB2], f32, name=f"costT{k}")
                nc.scalar.copy(out=costT, in_=psT)
                cmin = fp.tile([spt, 1], f32, name=f"cmin{k}")
                nc.vector.tensor_reduce(out=cmin, in_=costT, axis=AX.X,
                                        op=AT.min)
                oh8 = fp.tile([spt, NB2], f32, name=f"oh8{k}")
                nc.vector.tensor_scalar(out=oh8, in0=costT, scalar1=cmin,
                                        scalar2=None, op0=AT.is_equal)
                zc8 = fp.tile([spt, 1], f32, name=f"zc8{k}")
                nc.vector.memset(zc8, 0.0)
                pm8 = fp.tile([spt, NB2], f32, name=f"pm8{k}")
                nc.vector.tensor_tensor_scan(
                    out=pm8, data0=oh8, data1=_view(zc8, [[0, NB2]]),
                    initial=0.0, op0=AT.max, op1=AT.add)
                nc.vector.tensor_copy(out=oh8[:, 0:1], in_=pm8[:, 0:1])
                nc.vector.tensor_sub(out=oh8[:, 1:NB2], in0=pm8[:, 1:NB2],
                                     in1=pm8[:, 0:NB2 - 1])
                # ohcol[r] = oh8[n_r, b_r] via PE row-broadcast + masked accum
                psO = pp.tile([pk, NB2], f32, name=f"psO{k}")
                nc.tensor.matmul(psO, mask4t, oh8, start=True, stop=True)
                ohcol = fp.tile([pk, 1], f32, name=f"ohcol{k}")
                junkO = fp.tile([pk, NB2], f32, name=f"junkO{k}")
                nc.vector.scalar_tensor_tensor(
                    out=junkO, in0=psO, scalar=1.0, in1=mb32,
                    op0=AT.mult, op1=AT.mult, accum_out=ohcol)
                ohbd = fp.tile([pk, spt], f32, name=f"ohbd{k}")
                nc.vector.tensor_scalar(out=ohbd, in0=mask4, scalar1=ohcol,
                                        scalar2=None, op0=AT.mult)
                pj = pp.tile([spt, 3 * T], f32, name=f"pj{k}")
                nc.tensor.matmul(pj, ohbd, projs[k], start=True, stop=True)
                outt = fp.tile([spt, T, 3], f32, name=f"outt{k}")
                nc.scalar.copy(out=_view(outt, [[1, 3], [3, T]]),
                               in_=_view(pj, [[T, 3], [1, T]]))
                nc.sync.dma_start(out=out_d.ap()[k * spt:(k + 1) * spt],
                                  in_=outt)
            for k in range(ntiles):
                p = min(RT, rows - k * RT)
                r0 = k * RT

                rpt = wp.tile([p, 3, NP], f32, tag="rpt")
                nc.sync.dma_start(out=rpt, in_=rp_d.ap()[r0:r0 + p])
                mt = wp.tile([p, NP], f32, tag="mt")
                nc.gpsimd.dma_start(out=mt, in_=mk_d.ap()[r0:r0 + p])
                # trajectory of each row's sample, broadcast to its 32 rows
                tpb = wp.tile([p, 3, T], f32, tag="tpb")
                nc.scalar.dma_start(out=tpb, in_=bass.AP(
                    tensor=tj_d.ap().tensor, offset=k * spt * 3 * T,
                    ap=[[3 * T, spt], [0, NB2], [1, 3 * T]]))

                # --- segment data ---
                sv = wp.tile([p, 3, NSEG], f32, tag="sv")
                nc.vector.tensor_sub(out=sv, in0=rpt[:, :, 1:NP],
                                     in1=rpt[:, :, 0:NSEG])
                sm = wp.tile([p, NSEG], f32, tag="sm")
                nc.vector.tensor_mul(out=sm, in0=mt[:, 1:NP], in1=mt[:, 0:NSEG])
                sq3 = wp.tile([p, 3, NSEG], f32, tag="sq3")
                nc.vector.tensor_mul(out=sq3, in0=sv, in1=sv)
                sl2 = wp.tile([p, NSEG], f32, tag="sl2")
                nc.vector.tensor_reduce(out=sl2,
                                        in_=_view(sq3, [[1, NSEG], [NSEG, 3]]),
                                        axis=AX.X, op=AT.add)
                sl2m = wp.tile([p, NSEG], f32, tag="sl2m")
                nc.vector.tensor_mul(out=sl2m, in0=sl2, in1=sm)
                sl = wp.tile([p, NSEG], f32, tag="sl")
                eps2 = wp.tile([p, 1], f32, tag="eps2")
                nc.vector.memset(eps2, 1e-18)
                nc.scalar.activation(out=sl, in_=sl2m,
                                     func=mybir.ActivationFunctionType.Sqrt,
                                     bias=eps2)

                zc = wp.tile([p, 1], f32, tag="zc")
                nc.vector.memset(zc, 0.0)
                rscr = wp.tile([p, NSEG], f32, tag="rscr")
                # --- project p0 on all segments; entry_s via one-hot argmin ---
                tmp3 = wp.tile([p, 3, NSEG], f32, tag="tmp3")
                for c in range(3):
                    # (a_c - p0_c) * sv_c
                    nc.vector.scalar_tensor_tensor(
                        out=tmp3[:, c, :], in0=rpt[:, c, 0:NSEG],
                        scalar=tpb[:, c, 0:1], in1=sv[:, c, :],
                        op0=AT.subtract, op1=AT.mult)
                dotn = wp.tile([p, NSEG], f32, tag="dotn")
                nc.vector.tensor_reduce(out=dotn,
                                        in_=_view(tmp3, [[1, NSEG], [NSEG, 3]]),
                                        axis=AX.X, op=AT.add)
                svd = wp.tile([p, NSEG], f32, tag="svd")
                nc.vector.tensor_scalar(out=svd, in0=sl2, scalar1=1e-12,
                                        scalar2=None, op0=AT.max)
                rsvd = wp.tile([p, NSEG], f32, tag="rsvd")
                nc.vector.reciprocal_approx_accurate(out=rsvd, in_=svd,
                                                     scratch=rscr)
                t0 = wp.tile([p, NSEG], f32, tag="t0")
                nc.vector.tensor_mul(out=t0, in0=dotn, in1=rsvd)
                # t0 = min(max(-t0, 0), 1)
                nc.vector.tensor_scalar(out=t0, in0=t0, scalar1=-1.0,
                                        scalar2=0.0, op0=AT.mult, op1=AT.max)
                nc.vector.tensor_scalar(out=t0, in0=t0, scalar1=1.0,
                                        scalar2=None, op0=AT.min)
                s3 = wp.tile([p, 3, NSEG], f32, tag="s3")
                nc.vector.tensor_mul(out=s3, in0=sv,
                                     in1=_view(t0, [[0, 3], [1, NSEG]]))
                e3 = wp.tile([p, 3, NSEG], f32, tag="e3")
                for c in range(3):
                    # (a_c - p0_c) + t0*sv_c  (= q0_c - p0_c)
                    nc.vector.scalar_tensor_tensor(
                        out=e3[:, c, :], in0=rpt[:, c, 0:NSEG],
                        scalar=tpb[:, c, 0:1], in1=s3[:, c, :],
                        op0=AT.subtract, op1=AT.add)
                e3sq = wp.tile([p, 3, NSEG], f32, tag="e3sq")
                nc.vector.tensor_mul(out=e3sq, in0=e3, in1=e3)
                d2 = wp.tile([p, NSEG], f32, tag="d2")
                nc.vector.tensor_reduce(out=d2,
                                        in_=_view(e3sq, [[1, NSEG], [NSEG, 3]]),
                                        axis=AX.X, op=AT.add)
                d2m = wp.tile([p, NSEG], f32, tag="d2m")
                # d2m = d2 + (1-sm)*BIG  (sm is exactly 0/1)
                nc.vector.tensor_scalar(out=d2m, in0=sm, scalar1=1.0,
                                        scalar2=-BIG, op0=AT.subtract,
                                        op1=AT.mult)
                nc.vector.tensor_add(out=d2m, in0=d2m, in1=d2)
                dmin = wp.tile([p, 1], f32, tag="dmin")
                nc.vector.tensor_reduce(out=dmin, in_=d2m, axis=AX.X, op=AT.min)
                ohseg = wp.tile([p, NSEG], f32, tag="ohseg")
                nc.vector.tensor_scalar(out=ohseg, in0=d2m, scalar1=dmin,
                                        scalar2=None, op0=AT.is_equal)
                # keep only the FIRST hot via prefix-max diff (jnp.argmin ties)
                pmax = wp.tile([p, NSEG], f32, tag="pmax")
                nc.vector.tensor_tensor_scan(
                    out=pmax, data0=ohseg, data1=_view(zc, [[0, NSEG]]),
                    initial=0.0, op0=AT.max, op1=AT.add)
                nc.vector.tensor_copy(out=ohseg[:, 0:1], in_=pmax[:, 0:1])
                nc.vector.tensor_sub(out=ohseg[:, 1:NSEG], in0=pmax[:, 1:NSEG],
                                     in1=pmax[:, 0:NSEG - 1])
                cum = wp.tile([p, NP], f32, tag="cum")
                nc.vector.memset(cum[:, 0:1], 0.0)
                nc.vector.tensor_tensor_scan(
                    out=cum[:, 1:NP], data0=sl, data1=_view(zc, [[0, NSEG]]),
                    initial=0.0, op0=AT.add, op1=AT.add)
                total = cum[:, NP - 1:NP]
                # negated cumsum for the hinge stream (on ACT, frees DVE)
                cumneg = wp.tile([p, NSEG], f32, tag="cumneg")
                nc.scalar.activation(out=cumneg, in_=cum[:, 0:NSEG],
                                     func=mybir.ActivationFunctionType.Copy,
                                     scale=-1.0)
                rsl = wp.tile([p, NSEG], f32, tag="rsl")
                nc.vector.reciprocal_approx_accurate(out=rsl, in_=sl,
                                                     scratch=rscr)
                rslm = wp.tile([p, NSEG], f32, tag="rslm")
                nc.vector.tensor_mul(out=rslm, in0=rsl, in1=sm)
                wm = wp.tile([p, 3, NSEG], f32, tag="wm")
                nc.vector.tensor_mul(out=wm, in0=sv,
                                     in1=_view(rslm, [[0, 3], [1, NSEG]]))
                # dw[c,0] = wm[c,0]; dw[c,j] = wm[c,j]-wm[c,j-1]
                dw = wp.tile([p, 3, NSEG], f32, tag="dw")
                nc.scalar.copy(out=_view(dw, [[NSEG, 3]]),
                               in_=_view(wm, [[NSEG, 3]]))
                nc.vector.tensor_sub(
                    out=_view(dw, [[NSEG, 3], [1, NSEG - 1]], extra_off=1),
                    in0=_view(wm, [[NSEG, 3], [1, NSEG - 1]], extra_off=1),
                    in1=_view(wm, [[NSEG, 3], [1, NSEG - 1]]))

                es = wp.tile([p, NSEG], f32, tag="es")
                nc.vector.tensor_mul(out=es, in0=t0, in1=sl)
                nc.vector.tensor_add(out=es, in0=es, in1=cum[:, 0:NSEG])
                entry = wp.tile([p, 1], f32, tag="entry")
                junk0 = wp.tile([p, NSEG], f32, tag="junk0")
                nc.vector.scalar_tensor_tensor(
                    out=junk0, in0=ohseg, scalar=1.0, in1=es,
                    op0=AT.mult, op1=AT.mult, accum_out=entry)

                # --- base point rp[first valid segment] ---
                ohf = wp.tile([p, NSEG], f32, tag="ohf")
                nc.vector.tensor_copy(out=ohf[:, 0:1], in_=sm[:, 0:1])
                nc.vector.tensor_sub(out=ohf[:, 1:NSEG], in0=sm[:, 1:NSEG],
                                     in1=sm[:, 0:NSEG - 1])
                nc.vector.tensor_scalar(out=ohf, in0=ohf, scalar1=0.0,
                                        scalar2=None, op0=AT.max)
                base3 = wp.tile([p, 3], f32, tag="base3")
                for c in range(3):
                    nc.vector.scalar_tensor_tensor(
                        out=junk0, in0=ohf, scalar=1.0, in1=rpt[:, c, 0:NSEG],
                        op0=AT.mult, op1=AT.mult,
                        accum_out=base3[:, c:c + 1])

                # --- trajectory cumulative arc length + target_s ---
                td = wp.tile([p, 3, T - 1], f32, tag="td")
                nc.vector.tensor_sub(out=td, in0=tpb[:, :, 1:T],
                                     in1=tpb[:, :, 0:T - 1])
                td2 = wp.tile([p, 3, T - 1], f32, tag="td2")
                nc.vector.tensor_mul(out=td2, in0=td, in1=td)
                tl2 = wp.tile([p, T - 1], f32, tag="tl2")
                nc.vector.tensor_reduce(out=tl2,
                                        in_=_view(td2, [[1, T - 1], [T - 1, 3]]),
                                        axis=AX.X, op=AT.add)
                tl = wp.tile([p, T - 1], f32, tag="tl")
                nc.scalar.sqrt(out=tl, in_=tl2)
                tcum = wp.tile([p, T], f32, tag="tcum")
                nc.vector.memset(tcum[:, 0:1], 0.0)
                nc.vector.tensor_tensor_scan(
                    out=tcum[:, 1:T], data0=tl, data1=_view(zc, [[0, T - 1]]),
                    initial=0.0, op0=AT.add, op1=AT.add)
                target = wp.tile([p, T], f32, tag="target")
                nc.vector.scalar_tensor_tensor(
                    out=target, in0=tcum, scalar=entry,
                    in1=_view(total, [[0, T]]), op0=AT.add, op1=AT.min)
                nc.vector.tensor_scalar(out=target, in0=target, scalar1=0.0,
                                        scalar2=None, op0=AT.max)

                # --- main pass: fused hinge-scan, one DVE instr per t;
                # the last GP_T timesteps run on Pool (gpsimd) + ACT accum ---
                TD = T - GP_T
                E = wp.tile([p, 3, T], f32, tag="E")
                scrs = [wp.tile([p, 2 * 3 * NSEG], f32, name=f"scr{k}_{i}")
                        for i in range(2)]
                cn_b = _view(cumneg, [[0, 3], [1, NSEG]])
                emit_endgame(k - 1)
                if GP_T > 0:
                    Eg = wp.tile([p, 3, GP_T], f32, tag="Eg")
                    vts = [wp.tile([p, NSEG], f32, name=f"vt{k}_{i}")
                           for i in range(2)]
                    mcs = [wp.tile([p, NSEG], f32, name=f"mc{k}_{i}")
                           for i in range(6)]
                for t in range(T):
                    if t < TD:
                        pair = scrs[(t // 2) % 2]
                        nc.vector._custom_dve(
                            HINGE,
                            out=_view(pair, [[1, 3 * NSEG]],
                                      extra_off=(t % 2) * 3 * NSEG),
                            in0=cn_b, in1=dw,
                            s0=target[:, t:t + 1], s1=0.0)
                        if t % 2 == 1 or t == TD - 1:
                            nsl = 2 if t % 2 == 1 else 1
                            t0p = t - (nsl - 1)
                            nc.scalar.copy(
                                out=_view(E, [[1, nsl], [T, 3]],
                                          extra_off=t0p),
                                in_=_view(pair, [[3 * NSEG, nsl], [NSEG, 3]],
                                          extra_off=NSEG - 1))
                    else:
                        i = t - TD
                        vt = vts[i % 2]
                        nc.gpsimd.tensor_scalar(
                            out=vt, in0=cumneg, scalar1=target[:, t:t + 1],
                            scalar2=0.0, op0=AT.add, op1=AT.max)
                        for c in range(3):
                            mc = mcs[(3 * i + c) % 6]
                            nc.gpsimd.tensor_mul(out=mc, in0=vt,
                                                 in1=dw[:, c, :])
                            nc.scalar.activation(
                                out=mc, in_=mc,
                                func=mybir.ActivationFunctionType.Copy,
                                accum_out=Eg[:, c, i:i + 1])

                # page-end diffs + base -> proj
                proj = wp.tile([p, 3, T], f32, name=f"proj{k}", bufs=1)
                nc.vector.tensor_scalar(out=proj[:, 0, 0:TD], in0=E[:, 0, 0:TD],
                                        scalar1=base3[:, 0:1], scalar2=None,
                                        op0=AT.add)
                for c in (1, 2):
                    nc.vector.scalar_tensor_tensor(
                        out=proj[:, c, 0:TD], in0=E[:, c, 0:TD],
                        scalar=base3[:, c:c + 1], in1=E[:, c - 1, 0:TD],
                        op0=AT.add, op1=AT.subtract)
                if GP_T > 0:
                    for c in range(3):
                        nc.vector.tensor_scalar(
                            out=proj[:, c, TD:T], in0=Eg[:, c, :],
                            scalar1=base3[:, c:c + 1], scalar2=None,
                            op0=AT.add)
                projs.append(proj)

                # --- cost ---
                df2 = wp.tile([p, 3, T], f32, tag="df2")
                nc.vector._custom_dve(DIFFSQ, out=df2, in0=proj, in1=tpb,
                                      s0=0.0, s1=0.0)
                dd = wp.tile([p, T], f32, tag="dd")
                nc.vector.tensor_reduce(out=dd,
                                        in_=_view(df2, [[1, T], [T, 3]]),
                                        axis=AX.X, op=AT.add)
                dist = wp.tile([p, T], f32, tag="dist")
                cost = wp.tile([p, 1], f32, name=f"cost{k}", bufs=1)
                nc.scalar.activation(out=dist, in_=dd,
                                     func=mybir.ActivationFunctionType.Sqrt,
                                     accum_out=cost)
                costs.append(cost)

            for k in range(ntiles):
                emit_endgame(k)

    nc.compile()
    return nc


def marshal_inputs(selected_traj, road_points, road_mask):
    """Host-side layout marshaling (permutations/casts only): per-core input
    dicts with fwd+bwd branch rows and planar (xyz-major) layouts."""
    st = np.ascontiguousarray(selected_traj, dtype=np.float32)
    rp = np.ascontiguousarray(road_points, dtype=np.float32)
    rm = np.asarray(road_mask)

    rp_ext = np.concatenate([rp, rp[:, :, ::-1, :]], axis=1)        # [N,NB2,NP,3]
    rp_ext = np.ascontiguousarray(rp_ext.transpose(0, 1, 3, 2))     # [N,NB2,3,NP]
    mk_ext = np.concatenate([rm, rm[:, :, ::-1]], axis=1).astype(np.float32)
    tj = np.ascontiguousarray(st.transpose(0, 2, 1))                # [N,3,T]

    spt = RT // NB2
    m4 = np.zeros((RT, spt), dtype=np.float32)
    for s in range(spt):
        m4[s * NB2:(s + 1) * NB2, s] = 1.0
    m4t = np.ascontiguousarray(m4.T)
    mb = np.zeros((RT, NB2), dtype=np.float32)
    mb[np.arange(RT), np.arange(RT) % NB2] = 1.0

    in_maps = []
    for c in range(NCORES):
        s = slice(c * NS, (c + 1) * NS)
        in_maps.append({
            "rp": np.ascontiguousarray(rp_ext[s]).reshape(NS * NB2, 3, NP),
            "mk": np.ascontiguousarray(mk_ext[s]).reshape(NS * NB2, NP),
            "tj": np.ascontiguousarray(tj[s]),
            "m4": m4,
            "m4t": m4t,
            "mb": mb,
        })
    return in_maps


_NC = None


def kernel(selected_traj, road_points, road_mask):
    global _NC
    if _NC is None:
        _NC = build_nc()
    in_maps = marshal_inputs(selected_traj, road_points, road_mask)
    res = run_bass_kernel_spmd(_NC, in_maps, core_ids=list(range(NCORES)))
    out = np.concatenate([r["out"] for r in res.results], axis=0)
    return out.astype(np.float32)
